# revision 1
# baseline (speedup 1.0000x reference)
"""Trainium2 Bass kernel for BitNet multi-group-query attention.

Problem: nn_BitnetMultiGroupQueryAttention_41755672052100
  B=4, S=2048, E=2048, QH=16, KH=4, HD=128, KVE=512, fp32.

Key algebraic facts exploited (validated in numpy against the reference):
  * The reference einsum SUMS the query-head group axis, so the 4 query heads
    feeding each kv head are pre-summed in the (quantized) weights: the Q
    projection shrinks 4x.
  * softmax needs no max-subtraction here (scores are O(1)); the per-head
    normalizer is computed as a ones-matmul over probs, and per-token
    quantization scales fold into operands as per-partition scalars.
  * BitNet act/weight quantization produces small integers: projections are
    computed exactly with bf16 int-grid operands accumulating in fp32 PSUM.
  * Rounding uses the magic-constant trick ((x+1.5*2^23)-1.5*2^23 = fp32
    round-to-nearest-even); the act clip to [-127,127] never binds since
    scale=127/max|row|.

Layout strategy: activations are quantized in natural [token, feature] tiles,
bounced through a DRAM staging buffer as bf16 and DMA-transposed back in few
large xbar transfers (fp32 cannot use the DMA transpose path, bf16 can).
Attention runs entirely in the transposed [key, query] domain so probabilities
feed the PV matmul directly with no per-tile transposes.

Sharding: core c -> batch b=c//2, two 512-token query blocks ({0,3} even
half, {1,2} odd half; balanced causal work).  Every core computes k/v for the
full sequence of its batch.  The program is identical on all 8 cores (SPMD);
per-core behavior differs only through data (causal thresholds fed as input).
The low local block only attends to the first 1024 keys (true for blocks 0/1
on either half), which the program exploits statically.
"""

import os
import sys

for _p in ("/opt/trn_rl_repo", "/root/.axon_site/_ro/trn_rl_repo"):
    if os.path.isdir(_p) and _p not in sys.path:
        sys.path.insert(0, _p)
        break

import numpy as np

B, S, E = 4, 2048, 2048
QH, KH = 16, 4
HD, KVE = 128, 512
NCORES = 8
BLKS = [[0, 3], [1, 2]]        # global 512-token block ids per half
NT_Q = 1024                    # query tokens per core
MAGIC = 12582912.0             # 1.5 * 2**23 : fp32 RNE rounding constant
LN_EPS = 1e-5

_CACHE = {}


def _build(has_bv: bool):
    import concourse.bass as bass
    import concourse.tile as tile
    import concourse.mybir as mybir
    import concourse.bass_isa as bass_isa
    from concourse import bacc
    from concourse.masks import make_identity

    f32 = mybir.dt.float32
    bf16 = mybir.dt.bfloat16
    i32 = mybir.dt.int32
    ALU = mybir.AluOpType
    ACTF = mybir.ActivationFunctionType
    AX = mybir.AxisListType

    nc = bacc.Bacc(None, target_bir_lowering=False)

    # ---------------- DRAM I/O ----------------
    q_in = nc.dram_tensor("q_in", [NT_Q, E], f32, kind="ExternalInput").ap()
    k_in = nc.dram_tensor("k_in", [S, E], f32, kind="ExternalInput").ap()
    v_in = nc.dram_tensor("v_in", [S, E], f32, kind="ExternalInput").ap()
    wqT_d = nc.dram_tensor("wqT", [E, E], f32, kind="ExternalInput").ap()
    wkT_d = nc.dram_tensor("wkT", [E, KVE], f32, kind="ExternalInput").ap()
    wvT_d = nc.dram_tensor("wvT", [E, KVE], f32, kind="ExternalInput").ap()
    woT_d = nc.dram_tensor("woT", [KVE, E], f32, kind="ExternalInput").ap()
    bq_d = nc.dram_tensor("bq", [E], f32, kind="ExternalInput").ap()
    bv_d = nc.dram_tensor("bv", [KVE], f32, kind="ExternalInput").ap()
    bo_d = nc.dram_tensor("bo", [E], f32, kind="ExternalInput").ap()
    gamma_d = nc.dram_tensor("gamma", [KVE], f32, kind="ExternalInput").ap()
    beta_d = nc.dram_tensor("beta", [KVE], f32, kind="ExternalInput").ap()
    thr_d = nc.dram_tensor("thr", [2, 512], f32, kind="ExternalInput").ap()
    out_d = nc.dram_tensor("out", [NT_Q, E], f32, kind="ExternalOutput").ap()

    def bcast_ap(src_ap, parts=128):
        # DMA-replicate a free-only DRAM AP across `parts` partitions
        return bass.AP(
            tensor=src_ap.tensor,
            offset=src_ap.offset,
            ap=[[0, parts]] + list(src_ap.ap),
        )

    with tile.TileContext(nc) as tc:
      with tc.tile_pool(name="persist", bufs=1) as PP, \
           tc.tile_pool(name="wo_int", bufs=1) as WO, \
           tc.tile_pool(name="wq_int", bufs=1) as WIq, \
           tc.tile_pool(name="wkv_int", bufs=1) as WIkv, \
           tc.tile_pool(name="dram", bufs=1, space="DRAM") as DR:
        # ---------- small persistent constants ----------
        ones_col = PP.tile([128, 1], f32, tag="ones_col")
        nc.vector.memset(ones_col, 1.0)
        ones_row = PP.tile([1, 128], f32, tag="ones_row")
        nc.vector.memset(ones_row, 1.0)
        eps_col = PP.tile([128, 1], f32, tag="eps_col")
        nc.vector.memset(eps_col, LN_EPS)
        magic_col = PP.tile([128, 1], f32, tag="magic_col")
        nc.vector.memset(magic_col, MAGIC)
        ident = PP.tile([128, 128], f32, tag="ident")
        make_identity(nc, ident)
        sj_i = PP.tile([128, 16], i32, tag="sj_i")
        # sj[p, j] = p + 128*j  (global key index of partition p in s-tile j)
        nc.gpsimd.iota(sj_i, pattern=[[128, 16]], base=0, channel_multiplier=1)
        sj = PP.tile([128, 16], f32, tag="sj")
        nc.vector.tensor_copy(sj, sj_i)

        clip_k = PP.tile([128, 16], f32, tag="clip_k")
        clip_v = PP.tile([128, 16], f32, tag="clip_v")
        ck_all = PP.tile([128, 16], f32, tag="ck_all")
        cv_all = PP.tile([128, 16], f32, tag="cv_all")
        co_all = PP.tile([128, 8], f32, tag="co_all")

        # DRAM staging for bf16 transposes
        stage_k = DR.tile([S, E], bf16, tag="stage_k")
        stage_v = DR.tile([S, E], bf16, tag="stage_v")
        stage_q = DR.tile([NT_Q, E], bf16, tag="stage_q")
        stage_o = DR.tile([NT_Q, KVE], bf16, tag="stage_o")

        # ---------------- stage 0: weight quantization ----------------
        def finish_scale(acc, numel, tag):
            tot = PP.tile([128, 1], f32, tag=f"wtot_{tag}", name=f"wtot_{tag}")
            nc.gpsimd.partition_all_reduce(
                tot, acc, channels=128, reduce_op=bass_isa.ReduceOp.add
            )
            inv_col = PP.tile([128, 1], f32, tag=f"winv_{tag}", name=f"winv_{tag}")
            nc.vector.tensor_scalar(
                inv_col, tot, 1.0 / numel, 1e-5, op0=ALU.mult, op1=ALU.max
            )
            s_col = PP.tile([128, 1], f32, tag=f"ws_{tag}", name=f"ws_{tag}")
            nc.vector.reciprocal(s_col, inv_col)
            return s_col, inv_col

        def abs_acc(acc, tmp, t, first):
            if first:
                nc.vector.tensor_reduce(
                    acc, t, axis=AX.X, op=ALU.add, apply_absolute_value=True
                )
            else:
                nc.vector.tensor_reduce(
                    tmp, t, axis=AX.X, op=ALU.add, apply_absolute_value=True
                )
                nc.vector.tensor_add(acc, acc, tmp)

        def quant_tile(dst_bf16, src_f32, s_col, tmp_pool):
            # dst = clip(round(src * s), -1, 1) as bf16 (ternary ints)
            w = src_f32.shape[-1]
            for p0 in range(0, w, 512):
                pw = min(512, w - p0)
                t1 = tmp_pool.tile([128, 512], f32, tag="wq_t1")
                nc.scalar.activation(
                    out=t1[:, :pw], in_=src_f32[:, p0:p0 + pw],
                    func=ACTF.Identity, bias=magic_col, scale=s_col,
                )
                t2 = tmp_pool.tile([128, 512], f32, tag="wq_t2")
                nc.vector.tensor_scalar(
                    t2[:, :pw], t1[:, :pw], -MAGIC, 1.0, op0=ALU.add, op1=ALU.min
                )
                nc.gpsimd.tensor_scalar(
                    dst_bf16[:, p0:p0 + pw], t2[:, :pw], -1.0, None, op0=ALU.max
                )

        woqT = [WO.tile([128, E], bf16, tag=f"woq{c}", name=f"woq{c}")
                for c in range(4)]
        wqsumT = [WIq.tile([128, KVE], bf16, tag=f"wqsum{e}", name=f"wqsum{e}")
                  for e in range(16)]

        wkqT = [WIkv.tile([128, KVE], bf16, tag=f"wkq{e}", name=f"wkq{e}")
                for e in range(16)]
        wvqT = [WIkv.tile([128, KVE], bf16, tag=f"wvq{e}", name=f"wvq{e}")
                for e in range(16)]
        # --- Wk, Wv, Wo: small; keep fp32 resident in their scope ---
        with tc.tile_pool(name="wkvo_f32", bufs=1) as WF, \
             tc.tile_pool(name="wkvo_tmp", bufs=2) as WT:
            wk_t = [WF.tile([128, KVE], f32, tag=f"wkf{e}", name=f"wkf{e}")
                    for e in range(16)]
            wv_t = [WF.tile([128, KVE], f32, tag=f"wvf{e}", name=f"wvf{e}")
                    for e in range(16)]
            wo_t = [WF.tile([128, E], f32, tag=f"wof{c}", name=f"wof{c}")
                    for c in range(4)]
            for e in range(16):
                nc.sync.dma_start(out=wk_t[e], in_=wkT_d[e * 128:(e + 1) * 128, :])
                nc.gpsimd.dma_start(out=wv_t[e], in_=wvT_d[e * 128:(e + 1) * 128, :])
            for c in range(4):
                nc.gpsimd.dma_start(out=wo_t[c], in_=woT_d[c * 128:(c + 1) * 128, :])
            acc_k = PP.tile([128, 1], f32, tag="wacc_k")
            acc_v = PP.tile([128, 1], f32, tag="wacc_v")
            acc_o = PP.tile([128, 1], f32, tag="wacc_o")
            tmp_c = PP.tile([128, 1], f32, tag="wtmp_kvo")
            for e in range(16):
                abs_acc(acc_k, tmp_c, wk_t[e], e == 0)
            for e in range(16):
                abs_acc(acc_v, tmp_c, wv_t[e], e == 0)
            for c in range(4):
                abs_acc(acc_o, tmp_c, wo_t[c], c == 0)
            s_k, inv_swk = finish_scale(acc_k, float(KVE * E), "k")
            s_v, inv_swv = finish_scale(acc_v, float(KVE * E), "v")
            s_o, inv_swo = finish_scale(acc_o, float(E * KVE), "o")
            for e in range(16):
                quant_tile(wkqT[e], wk_t[e], s_k, WT)
                quant_tile(wvqT[e], wv_t[e], s_v, WT)
            for c in range(4):
                quant_tile(woqT[c], wo_t[c], s_o, WT)

        # --- Wq: two streaming passes, interleaved piecewise into the
        # key/value chunk loops so loads+quant overlap K/V projections ---
        wq_state = {}

        def wq_pass1_piece(WL, i):
            if i == 0:
                wq_state["acc"] = PP.tile([128, 1], f32, tag="wacc_q",
                                           name="wacc_q")
                wq_state["tmpc"] = PP.tile([128, 1], f32, tag="wtmp_q",
                                           name="wtmp_q")
            for e in range(4 * i, 4 * i + 4):
                t = WL.tile([128, E], f32, tag="wq_load")
                (nc.sync if e % 2 else nc.scalar).dma_start(
                    out=t, in_=wqT_d[e * 128:(e + 1) * 128, :])
                abs_acc(wq_state["acc"], wq_state["tmpc"], t, e == 0)
            if i == 3:
                s_q, inv_swq = finish_scale(wq_state["acc"], float(E * E), "q")
                wq_state["s_q"] = s_q
                wq_state["inv_swq"] = inv_swq

        def wq_pass2_piece(WL, WT, i):
            s_q = wq_state["s_q"]
            for e in range(4 * i, 4 * i + 4):
                t = WL.tile([128, E], f32, tag="wq_load")
                (nc.scalar if e % 2 else nc.sync).dma_start(
                    out=t, in_=wqT_d[e * 128:(e + 1) * 128, :])
                for h in range(KH):
                    t1 = WT.tile([128, 512], f32, tag="wq_t1")
                    nc.scalar.activation(
                        out=t1, in_=t[:, h * 512:(h + 1) * 512],
                        func=ACTF.Identity, bias=magic_col, scale=s_q,
                    )
                    t2 = WT.tile([128, 512], f32, tag="wq_t2")
                    nc.vector.tensor_scalar(
                        t2, t1, -MAGIC, 1.0, op0=ALU.add, op1=ALU.min
                    )
                    wqp = WT.tile([128, 512], bf16, tag="wq_p")
                    nc.gpsimd.tensor_scalar(wqp, t2, -1.0, None, op0=ALU.max)
                    tmps = WT.tile([128, HD], f32, tag="wq_sumf")
                    nc.vector.tensor_reduce(
                        tmps,
                        wqp.rearrange("p (g d) -> p d g", g=4, d=HD),
                        axis=AX.X,
                        op=ALU.add,
                    )
                    nc.gpsimd.tensor_copy(
                        wqsumT[e][:, h * 128:(h + 1) * 128], tmps
                    )

        # summed q bias, pre-scaled by 1/128:
        bq_sb = PP.tile([128, 16], f32, tag="bq_sb")
        nc.sync.dma_start(out=bq_sb, in_=bq_d.rearrange("(j d) -> d j", d=128))
        bqsum = PP.tile([128, KH], f32, tag="bqsum")
        nc.vector.tensor_reduce(
            bqsum,
            bq_sb.rearrange("p (h g) -> p h g", h=KH, g=4),
            axis=AX.X,
            op=ALU.add,
        )
        nc.vector.tensor_scalar_mul(bqsum, bqsum, 1.0 / 128.0)

        # ======= activation persistents (after weight fp32 pools closed) ====
        with tc.tile_pool(name="act_p1", bufs=1) as A1:
            kT = [A1.tile([128, S], bf16, tag=f"kT{h}", name=f"kT{h}")
                  for h in range(KH)]                  # [d, s] int-grid
            vS = [A1.tile([128, KVE], f32, tag=f"v{j}", name=f"v{j}")
                  for j in range(16)]                  # [s, dv] cv-folded
            qT = [A1.tile([128, NT_Q], bf16, tag=f"qT{h}", name=f"qT{h}")
                  for h in range(KH)]                  # [d, n] cq-folded
            if has_bv:
                bv_bc = A1.tile([128, KVE], f32, tag="bv_bc")
                nc.gpsimd.dma_start(out=bv_bc, in_=bcast_ap(bv_d))

            # ------------- stage 1: act quant + transpose + projections -----
            CHUNK = 512
            QW = 512   # column piece for quantize elementwise ops

            def act_quant_tile(xtile, t2_dst, TQ, fold_col=None, save_clip=None,
                               save_c=None, c_mults=None, t1_on_act=True):
                """Quantize one [128, W] token tile into t2_dst (bf16 ints,
                optionally * fold_col).  save_clip/save_c: [128,1] dsts."""
                w = xtile.shape[-1]
                if not callable(t2_dst):
                    _dst_ap = t2_dst
                    t2_dst = lambda c0: _dst_ap[:, c0:min(c0 + QW, w)]
                mx = TQ.tile([128, 1], f32, tag="aq_mx")
                nc.vector.tensor_reduce(
                    mx, xtile, axis=AX.X, op=ALU.max, apply_absolute_value=True
                )
                clip = TQ.tile([128, 1], f32, tag="aq_clip")
                nc.vector.tensor_scalar(clip, mx, 1e-5, None, op0=ALU.max)
                if save_clip is not None:
                    nc.gpsimd.tensor_copy(save_clip, clip)
                sx = TQ.tile([128, 1], f32, tag="aq_sx")
                nc.vector.reciprocal(sx, clip)
                nc.vector.tensor_scalar(sx, sx, 127.0, None, op0=ALU.mult)
                if save_c is not None:
                    nc.vector.tensor_scalar(
                        save_c, clip, c_mults[0], c_mults[1],
                        op0=ALU.mult, op1=ALU.mult,
                    )
                for c0 in range(0, w, QW):
                    cw = min(QW, w - c0)
                    t1 = TQ.tile([128, QW], f32, tag="aq_t1")
                    if t1_on_act:
                        nc.scalar.activation(
                            out=t1[:, :cw], in_=xtile[:, c0:c0 + cw],
                            func=ACTF.Identity, bias=magic_col, scale=sx,
                        )
                    else:
                        nc.vector.tensor_scalar(
                            t1[:, :cw], xtile[:, c0:c0 + cw], sx, MAGIC,
                            op0=ALU.mult, op1=ALU.add,
                        )
                    if fold_col is not None:
                        nc.gpsimd.tensor_scalar(
                            t2_dst(c0)[:, :cw], t1[:, :cw], -MAGIC, fold_col,
                            op0=ALU.add, op1=ALU.mult,
                        )
                    else:
                        nc.gpsimd.tensor_scalar(
                            t2_dst(c0)[:, :cw], t1[:, :cw], -MAGIC, None,
                            op0=ALU.add,
                        )

            def quant_chunk(AL, AQ, AC, src_dram, stage, c0, kind, teng):
                """Load+quantize CHUNK tokens at row c0, bounce via DRAM
                stage, return transposed chunk [128, 16, CHUNK] bf16."""
                for ti in range(CHUNK // 128):
                    tok0 = c0 + ti * 128
                    xt = AL.tile([128, E], f32, tag="aload")
                    (nc.sync if ti % 2 else nc.scalar).dma_start(
                        out=xt, in_=src_dram[tok0:tok0 + 128, :])
                    jt = tok0 // 128
                    t2 = AQ.tile([128, E], bf16, tag="aq_t2")
                    t2_dst = t2
                    if kind == "q":
                        cq = AQ.tile([128, 1], f32, tag="aq_cq")
                        act_quant_tile(
                            xt, t2_dst, AQ, fold_col=cq, save_c=cq,
                            c_mults=(wq_state["inv_swq"],
                                     1.0 / (127.0 * 128.0)),
                            t1_on_act=False,
                        )
                    elif kind == "k":
                        act_quant_tile(xt, t2_dst, AQ,
                                       save_clip=clip_k[:, jt:jt + 1],
                                       t1_on_act=False)
                    else:
                        act_quant_tile(xt, t2_dst, AQ,
                                       save_clip=clip_v[:, jt:jt + 1],
                                       t1_on_act=False)
                    (nc.scalar if ti % 2 else nc.sync).dma_start(
                        out=stage[tok0:tok0 + 128, :], in_=t2
                    )
                chunk = AC.tile([128, 16, CHUNK], bf16, tag="chunk")
                # one xbar transfer: chunk[p, e, t] = stage[c0+t, e*128+p]
                teng.dma_start_transpose(
                    out=chunk, in_=stage[c0:c0 + CHUNK, :]
                )
                return chunk

            # ---- key + value (weight-independent quant; projections use the
            # small weights which quantize quickly).  Wq pools enclose this
            # scope; Wq ops are emitted last so they fill engine gaps. ----
            with tc.tile_pool(name="wq_load", bufs=2) as WQL, \
                 tc.tile_pool(name="wq_tmp", bufs=1) as WQT, \
                 tc.tile_pool(name="aload", bufs=2) as AL, \
                 tc.tile_pool(name="aquant", bufs=2) as AQ, \
                 tc.tile_pool(name="achunk", bufs=2) as AC, \
                 tc.tile_pool(name="proj_psum", bufs=2, space="PSUM") as PJ, \
                 tc.tile_pool(name="q_psum", bufs=1, space="PSUM") as QJ:
                for ci in range(S // CHUNK):
                    c0 = ci * CHUNK
                    chunk = quant_chunk(AL, AQ, AC, k_in, stage_k, c0, "k",
                                        nc.scalar)
                    for h in range(KH):
                        ps = PJ.tile([128, CHUNK], f32, tag="proj_ps")
                        for e in range(16):
                            nc.tensor.matmul(
                                ps,
                                lhsT=wkqT[e][:, h * 128:(h + 1) * 128],
                                rhs=chunk[:, e, :],
                                start=(e == 0),
                                stop=(e == 15),
                            )
                        nc.vector.tensor_copy(kT[h][:, c0:c0 + CHUNK], ps)
                    wq_pass1_piece(WQL, ci)

                q_hold = {}

                def emit_query_a(qi):
                    # quant + stage + transpose + first 8 e-tiles of Q-proj
                    # (wqsumT[0:8] are written by wq_pass2 pieces 0-1)
                    c0 = qi * CHUNK
                    chunk = quant_chunk(AL, AQ, AC, q_in, stage_q, c0, "q",
                                        nc.scalar)
                    pss = []
                    for h in range(KH):
                        ps = QJ.tile([128, CHUNK], f32, tag=f"proj_q{h}",
                                     name=f"proj_q{h}_{qi}")
                        for e in range(8):
                            nc.tensor.matmul(
                                ps,
                                lhsT=wqsumT[e][:, h * 128:(h + 1) * 128],
                                rhs=chunk[:, e, :],
                                start=(e == 0),
                                stop=False,
                            )
                        pss.append(ps)
                    q_hold[qi] = (chunk, pss)

                def emit_query_b(qi):
                    # remaining 8 e-tiles + epilogue (needs all wqsumT)
                    c0 = qi * CHUNK
                    chunk, pss = q_hold.pop(qi)
                    for h in range(KH):
                        ps = pss[h]
                        for e in range(8, 16):
                            nc.tensor.matmul(
                                ps,
                                lhsT=wqsumT[e][:, h * 128:(h + 1) * 128],
                                rhs=chunk[:, e, :],
                                start=False,
                                stop=(e == 15),
                            )
                        nc.scalar.activation(
                            out=qT[h][:, c0:c0 + CHUNK],
                            in_=ps,
                            func=ACTF.Identity,
                            bias=bqsum[:, h:h + 1],
                            scale=1.0,
                        )

                for ci in range(S // CHUNK):
                    c0 = ci * CHUNK
                    chunk = quant_chunk(AL, AQ, AC, v_in, stage_v, c0, "v",
                                        nc.sync)
                    # cv for this chunk's 4 token tiles (clips just written)
                    j0 = c0 // 128
                    nc.vector.tensor_scalar(
                        cv_all[:, j0:j0 + 4], clip_v[:, j0:j0 + 4],
                        inv_swv, 1.0 / 127.0, op0=ALU.mult, op1=ALU.mult,
                    )
                    for ti in range(CHUNK // 128):
                        jt = (c0 + ti * 128) // 128
                        ps = PJ.tile([128, KVE], f32, tag="proj_ps_v")
                        for e in range(16):
                            nc.tensor.matmul(
                                ps,
                                lhsT=chunk[:, e, ti * 128:(ti + 1) * 128],
                                rhs=wvqT[e],
                                start=(e == 0),
                                stop=(e == 15),
                            )
                        nc.vector.tensor_scalar(
                            vS[jt], ps, cv_all[:, jt:jt + 1], None, op0=ALU.mult
                        )
                        if has_bv:
                            nc.vector.tensor_add(vS[jt], vS[jt], bv_bc)
                    wq_pass2_piece(WQL, WQT, ci)
                    if ci == 1:
                        emit_query_a(0)
                    elif ci == 3:
                        emit_query_b(0)
                        emit_query_a(1)
                        emit_query_b(1)

            # ck columns for the exp stage
            nc.vector.tensor_scalar(
                ck_all, clip_k, inv_swk, 1.0 / 127.0, op0=ALU.mult, op1=ALU.mult
            )

            with tc.tile_pool(name="act_p2", bufs=1) as A2:
                xT = [A2.tile([128, NT_Q], f32, tag=f"xT{h}", name=f"xT{h}")
                      for h in range(KH)]              # [c, n]
                xqoT = A2.tile([128, 4, NT_Q], bf16, tag="xqoT")
                thr_bc = [A2.tile([128, 512], f32, tag=f"thr{lb}", name=f"thr{lb}")
                          for lb in range(2)]
                for lb in range(2):
                    nc.gpsimd.dma_start(out=thr_bc[lb], in_=bcast_ap(thr_d[lb]))
                gamma_bc = A2.tile([128, KVE], f32, tag="gamma_bc")
                beta_bc = A2.tile([128, KVE], f32, tag="beta_bc")
                bo_bc = A2.tile([128, E], f32, tag="bo_bc")
                nc.gpsimd.dma_start(out=gamma_bc, in_=bcast_ap(gamma_d))
                nc.gpsimd.dma_start(out=beta_bc, in_=bcast_ap(beta_d))
                nc.gpsimd.dma_start(out=bo_bc, in_=bcast_ap(bo_d))

                # ---------------- stage 2: attention ----------------
                # local block 0 is one of global blocks {0,1}: keys < 1024
                NJ = [8, 16]
                with tc.tile_pool(name="amask", bufs=1) as MP, \
                     tc.tile_pool(name="aprobs", bufs=4) as PB, \
                     tc.tile_pool(name="azrow", bufs=2) as ZR, \
                     tc.tile_pool(name="sim_psum", bufs=2, space="PSUM") as SP_, \
                     tc.tile_pool(name="x_psum", bufs=2, space="PSUM") as XP, \
                     tc.tile_pool(name="z_psum", bufs=2, space="PSUM") as ZP, \
                     tc.tile_pool(name="b_psum", bufs=1, space="PSUM") as BP:
                    for lb in range(2):
                        nj = NJ[lb]
                        masks = [MP.tile([128, 512], bf16, tag=f"mask{j}",
                                         name=f"mask{j}_{lb}")
                                 for j in range(nj)]
                        for j in range(nj):
                            # mask[p, n] = (thr[lb, n] >= p + 128*j)
                            nc.vector.tensor_scalar(
                                masks[j], thr_bc[lb], sj[:, j:j + 1], None,
                                op0=ALU.is_ge,
                            )
                        for h in range(KH):
                            ps_x = XP.tile([128, 512], f32, tag="ps_x")
                            ps_z = ZP.tile([1, 512], f32, tag="ps_z")
                            for j in range(nj):
                                ps_s = SP_.tile([128, 512], f32, tag="ps_s")
                                nc.tensor.matmul(
                                    ps_s,
                                    lhsT=kT[h][:, j * 128:(j + 1) * 128],
                                    rhs=qT[h][:, lb * 512:(lb + 1) * 512],
                                    start=True,
                                    stop=True,
                                )
                                probs = PB.tile([128, 512], f32, tag="probs")
                                nc.scalar.activation(
                                    out=probs, in_=ps_s, func=ACTF.Exp,
                                    scale=ck_all[:, j:j + 1],
                                )
                                nc.gpsimd.tensor_mul(probs, probs, masks[j])
                                nc.tensor.matmul(
                                    ps_x,
                                    lhsT=vS[j][:, h * 128:(h + 1) * 128],
                                    rhs=probs,
                                    start=(j == 0),
                                    stop=(j == nj - 1),
                                )
                                nc.tensor.matmul(
                                    ps_z,
                                    lhsT=ones_col,
                                    rhs=probs,
                                    start=(j == 0),
                                    stop=(j == nj - 1),
                                )
                            invz = ZR.tile([1, 512], f32, tag="invz")
                            nc.vector.reciprocal(invz, ps_z)
                            ps_b = BP.tile([128, 512], f32, tag="ps_b")
                            nc.tensor.matmul(ps_b, lhsT=ones_row, rhs=invz,
                                             start=True, stop=True)
                            invz_bc = ZR.tile([128, 512], f32, tag="invz_bc")
                            nc.vector.tensor_copy(invz_bc, ps_b)
                            nc.vector.tensor_mul(
                                xT[h][:, lb * 512:(lb + 1) * 512], ps_x, invz_bc
                            )

                # ---------------- stage 3: layernorm + out quant ------------
                with tc.tile_pool(name="ln", bufs=2) as LN, \
                     tc.tile_pool(name="t_psum", bufs=2, space="PSUM") as TP:
                    for tb in range(NT_Q // 128):
                        xt = LN.tile([128, KVE], f32, tag="ln_x")
                        for c in range(4):
                            ps_t = TP.tile([128, 128], f32, tag="ps_t")
                            nc.tensor.transpose(
                                ps_t, xT[c][:, tb * 128:(tb + 1) * 128], ident
                            )
                            nc.vector.tensor_copy(
                                xt[:, c * 128:(c + 1) * 128], ps_t)
                        stats = LN.tile([128, 6], f32, tag="ln_stats")
                        nc.vector.bn_stats(out=stats, in_=xt)
                        mv = LN.tile([128, 2], f32, tag="ln_mv")
                        nc.vector.bn_aggr(out=mv, in_=stats)
                        sd = LN.tile([128, 1], f32, tag="ln_sd")
                        nc.scalar.activation(
                            out=sd, in_=mv[:, 1:2], func=ACTF.Sqrt, bias=eps_col,
                        )
                        rstd = LN.tile([128, 1], f32, tag="ln_rstd")
                        nc.vector.reciprocal(rstd, sd)
                        xn = LN.tile([128, KVE], f32, tag="ln_xn")
                        nc.vector.tensor_scalar(
                            xn, xt, mv[:, 0:1], rstd,
                            op0=ALU.subtract, op1=ALU.mult,
                        )
                        nc.vector.tensor_mul(xn, xn, gamma_bc)
                        nc.vector.tensor_add(xn, xn, beta_bc)
                        xqo = LN.tile([128, KVE], bf16, tag="ln_xqo")
                        act_quant_tile(
                            xn, xqo, LN, save_c=co_all[:, tb:tb + 1],
                            c_mults=(inv_swo, 1.0 / 127.0),
                        )
                        nc.scalar.dma_start(
                            out=stage_o[tb * 128:(tb + 1) * 128, :], in_=xqo
                        )
                    nc.sync.dma_start_transpose(out=xqoT, in_=stage_o[:, :])

                # ---------------- stage 4: output projection ----------------
                with tc.tile_pool(name="osb", bufs=3) as OS, \
                     tc.tile_pool(name="o_psum", bufs=2, space="PSUM") as OP:
                    for tb in range(NT_Q // 128):
                        for eb in range(4):
                            ps_o = OP.tile([128, 512], f32, tag="ps_o")
                            for c in range(4):
                                nc.tensor.matmul(
                                    ps_o,
                                    lhsT=xqoT[:, c, tb * 128:(tb + 1) * 128],
                                    rhs=woqT[c][:, eb * 512:(eb + 1) * 512],
                                    start=(c == 0),
                                    stop=(c == 3),
                                )
                            ot = OS.tile([128, 512], f32, tag="o_t")
                            nc.vector.tensor_scalar(
                                ot, ps_o, co_all[:, tb:tb + 1], None,
                                op0=ALU.mult,
                            )
                            nc.gpsimd.tensor_add(
                                ot, ot, bo_bc[:, eb * 512:(eb + 1) * 512]
                            )
                            nc.sync.dma_start(
                                out=out_d[tb * 128:(tb + 1) * 128,
                                          eb * 512:(eb + 1) * 512],
                                in_=ot,
                            )

    nc.compile()
    return nc


def _get_nc(has_bv):
    key = ("nc", has_bv)
    if key not in _CACHE:
        _CACHE[key] = _build(has_bv)
    return _CACHE[key]


def kernel(query, key, value, Wq, bq, Wk, bk, Wv, bv, Wo, bo, gamma, beta):
    from concourse.bass_utils import run_bass_kernel_spmd

    query = np.ascontiguousarray(query, np.float32)
    key = np.ascontiguousarray(key, np.float32)
    value = np.ascontiguousarray(value, np.float32)
    wqT = np.ascontiguousarray(np.asarray(Wq, np.float32).T)
    wkT = np.ascontiguousarray(np.asarray(Wk, np.float32).T)
    wvT = np.ascontiguousarray(np.asarray(Wv, np.float32).T)
    woT = np.ascontiguousarray(np.asarray(Wo, np.float32).T)
    bq = np.ascontiguousarray(bq, np.float32)
    bv_ = np.ascontiguousarray(bv, np.float32)
    bo = np.ascontiguousarray(bo, np.float32)
    gamma = np.ascontiguousarray(gamma, np.float32)
    beta = np.ascontiguousarray(beta, np.float32)

    has_bv = bool(np.any(bv_ != 0))
    nc = _get_nc(has_bv)

    in_maps = []
    for c in range(NCORES):
        b, half = c // 2, c % 2
        blocks = BLKS[half]
        q_rows = np.concatenate(
            [query[b, blk * 512:(blk + 1) * 512, :] for blk in blocks], axis=0
        )
        thr = np.stack(
            [blk * 512 + np.arange(512, dtype=np.float32) for blk in blocks]
        )
        in_maps.append({
            "q_in": np.ascontiguousarray(q_rows),
            "k_in": key[b],
            "v_in": value[b],
            "wqT": wqT, "wkT": wkT, "wvT": wvT, "woT": woT,
            "bq": bq, "bv": bv_, "bo": bo,
            "gamma": gamma, "beta": beta,
            "thr": np.ascontiguousarray(thr),
        })

    res = run_bass_kernel_spmd(nc, in_maps, core_ids=list(range(NCORES)))
    _CACHE["last_result"] = res

    out = np.zeros((B, S, E), np.float32)
    for c in range(NCORES):
        b, half = c // 2, c % 2
        blocks = BLKS[half]
        o = res.results[c]["out"]
        for i, blk in enumerate(blocks):
            out[b, blk * 512:(blk + 1) * 512, :] = o[i * 512:(i + 1) * 512, :]
    return out



# revision 8
# speedup vs baseline: 3.4756x; 3.4756x over previous
"""Trainium2 Bass kernel for BitNet multi-group-query attention.

Problem: nn_BitnetMultiGroupQueryAttention_41755672052100
  B=4, S=2048, E=2048, QH=16, KH=4, HD=128, KVE=512, fp32.

Key algebraic facts exploited (validated in numpy against the reference):
  * The reference einsum SUMS the query-head group axis, so the 4 query heads
    feeding each kv head are pre-summed in the (quantized) weights: the Q
    projection shrinks 4x.
  * softmax needs no max-subtraction here (scores are O(1)); the per-head
    normalizer is computed as a ones-matmul over probs, and per-token
    quantization scales fold into operands as per-partition scalars.
  * BitNet act/weight quantization produces small integers: projections are
    computed exactly with bf16 int-grid operands accumulating in fp32 PSUM.
  * Rounding uses the magic-constant trick ((x+1.5*2^23)-1.5*2^23 = fp32
    round-to-nearest-even); the act clip to [-127,127] never binds since
    scale=127/max|row|.

Layout strategy: activations are quantized in natural [token, feature] tiles,
bounced through a DRAM staging buffer as bf16 and DMA-transposed back in few
large xbar transfers (fp32 cannot use the DMA transpose path, bf16 can).
Attention runs entirely in the transposed [key, query] domain so probabilities
feed the PV matmul directly with no per-tile transposes.  Probabilities, V
and attention outputs are held in bf16 so every matmul runs at the PE's
full bf16 rate (fp32 matmuls cost 4 cycles/row).

Sharding: core c -> batch b=c//2, two 512-token query blocks ({0,3} even
half, {1,2} odd half; balanced causal work).  Every core computes k/v for the
full sequence of its batch.  The program is identical on all 8 cores (SPMD);
per-core behavior differs only through data (causal thresholds fed as input).
The low local block only attends to the first 1024 keys (true for blocks 0/1
on either half), which the program exploits statically.  The high block needs
no mask for the first 1024 keys (always visible), halving mask multiplies.
"""

import os
import sys

for _p in ("/opt/trn_rl_repo", "/root/.axon_site/_ro/trn_rl_repo"):
    if os.path.isdir(_p) and _p not in sys.path:
        sys.path.insert(0, _p)
        break

import numpy as np

B, S, E = 4, 2048, 2048
QH, KH = 16, 4
HD, KVE = 128, 512
NCORES = 8
BLKS = [[0, 3], [1, 2]]        # global 512-token block ids per half
NT_Q = 1024                    # query tokens per core
MAGIC = 12582912.0             # 1.5 * 2**23 : fp32 RNE rounding constant
LN_EPS = 1e-5

_CACHE = {}


def _build(has_bq: bool, has_bv: bool, has_bo: bool, has_ln: bool):
    import concourse.bass as bass
    import concourse.tile as tile
    import concourse.mybir as mybir
    import concourse.bass_isa as bass_isa
    from concourse import bacc
    from concourse.masks import make_identity

    f32 = mybir.dt.float32
    bf16 = mybir.dt.bfloat16
    i32 = mybir.dt.int32
    ALU = mybir.AluOpType
    ACTF = mybir.ActivationFunctionType
    AX = mybir.AxisListType

    nc = bacc.Bacc(None, target_bir_lowering=False)

    # ---------------- DRAM I/O ----------------
    q_in = nc.dram_tensor("q_in", [NT_Q, E], f32, kind="ExternalInput").ap()
    k_in = nc.dram_tensor("k_in", [S, E], f32, kind="ExternalInput").ap()
    v_in = nc.dram_tensor("v_in", [S, E], f32, kind="ExternalInput").ap()
    wqT_d = nc.dram_tensor("wqT", [E, E], f32, kind="ExternalInput").ap()
    wkT_d = nc.dram_tensor("wkT", [E, KVE], f32, kind="ExternalInput").ap()
    wvT_d = nc.dram_tensor("wvT", [E, KVE], f32, kind="ExternalInput").ap()
    woT_d = nc.dram_tensor("woT", [KVE, E], f32, kind="ExternalInput").ap()
    bq_d = nc.dram_tensor("bq", [E], f32, kind="ExternalInput").ap()
    bv_d = nc.dram_tensor("bv", [KVE], f32, kind="ExternalInput").ap()
    bo_d = nc.dram_tensor("bo", [E], f32, kind="ExternalInput").ap()
    gamma_d = nc.dram_tensor("gamma", [KVE], f32, kind="ExternalInput").ap()
    beta_d = nc.dram_tensor("beta", [KVE], f32, kind="ExternalInput").ap()
    thr_d = nc.dram_tensor("thr", [2, 512], f32, kind="ExternalInput").ap()
    out_d = nc.dram_tensor("out", [NT_Q, E], f32, kind="ExternalOutput").ap()

    def bcast_ap(src_ap, parts=128):
        # DMA-replicate a free-only DRAM AP across `parts` partitions
        return bass.AP(
            tensor=src_ap.tensor,
            offset=src_ap.offset,
            ap=[[0, parts]] + list(src_ap.ap),
        )

    with tile.TileContext(nc) as tc:
      with tc.tile_pool(name="persist", bufs=1) as PP, \
           tc.tile_pool(name="acts", bufs=1) as A1, \
           tc.tile_pool(name="wo_int", bufs=1) as WO, \
           tc.tile_pool(name="dram", bufs=1, space="DRAM") as DR:
        # ---------- small persistent constants ----------
        ones_col = PP.tile([128, 1], bf16, tag="ones_col")
        nc.vector.memset(ones_col, 1.0)
        ones_row = PP.tile([1, 128], f32, tag="ones_row")
        nc.vector.memset(ones_row, 1.0)
        eps_col = PP.tile([128, 1], f32, tag="eps_col")
        nc.vector.memset(eps_col, LN_EPS)
        magic_col = PP.tile([128, 1], f32, tag="magic_col")
        nc.vector.memset(magic_col, MAGIC)
        ident = PP.tile([128, 128], bf16, tag="ident")
        make_identity(nc, ident)
        sj_i = PP.tile([128, 16], i32, tag="sj_i")
        # sj[p, j] = p + 128*j  (global key index of partition p in s-tile j)
        nc.gpsimd.iota(sj_i, pattern=[[128, 16]], base=0, channel_multiplier=1)
        sj = PP.tile([128, 16], f32, tag="sj")
        nc.vector.tensor_copy(sj, sj_i)

        clip_k = PP.tile([128, 16], f32, tag="clip_k")
        clip_v = PP.tile([128, 16], f32, tag="clip_v")
        ck_all = PP.tile([128, 16], f32, tag="ck_all")
        cv_all = PP.tile([128, 16], f32, tag="cv_all")
        co_all = PP.tile([128, 8], f32, tag="co_all")

        # DRAM staging for bf16 transposes
        stage_k = DR.tile([S, E], bf16, tag="stage_k")
        stage_v = DR.tile([S, E], bf16, tag="stage_v")
        stage_q = DR.tile([NT_Q, E], bf16, tag="stage_q")
        stage_o = DR.tile([NT_Q, KVE], bf16, tag="stage_o")

        # persistent activations
        kT = [A1.tile([128, S], bf16, tag=f"kT{h}", name=f"kT{h}")
              for h in range(KH)]                  # [d, s] int-grid
        vS = [A1.tile([128, KVE], bf16, tag=f"v{j}", name=f"v{j}")
              for j in range(16)]                  # [s, dv] cv-folded, bf16
        qT = [A1.tile([128, NT_Q], bf16, tag=f"qT{h}", name=f"qT{h}")
              for h in range(KH)]                  # [d, n] cq-folded

        woqT = [WO.tile([128, E], bf16, tag=f"woq{c}", name=f"woq{c}")
                for c in range(4)]

        # ---------------- helpers ----------------
        def finish_scale(acc, numel, tag):
            tot = PP.tile([128, 1], f32, tag=f"wtot_{tag}", name=f"wtot_{tag}")
            nc.gpsimd.partition_all_reduce(
                tot, acc, channels=128, reduce_op=bass_isa.ReduceOp.add
            )
            inv_col = PP.tile([128, 1], f32, tag=f"winv_{tag}", name=f"winv_{tag}")
            nc.vector.tensor_scalar(
                inv_col, tot, 1.0 / numel, 1e-5, op0=ALU.mult, op1=ALU.max
            )
            s_col = PP.tile([128, 1], f32, tag=f"ws_{tag}", name=f"ws_{tag}")
            nc.vector.reciprocal(s_col, inv_col)
            return s_col, inv_col

        def abs_acc(acc, tmp, t, first):
            if first:
                nc.vector.tensor_reduce(
                    acc, t, axis=AX.X, op=ALU.add, apply_absolute_value=True
                )
            else:
                nc.vector.tensor_reduce(
                    tmp, t, axis=AX.X, op=ALU.add, apply_absolute_value=True
                )
                nc.vector.tensor_add(acc, acc, tmp)

        def quant_tile(dst_bf16, src_f32, s_col, tmp_pool, salt=0):
            # dst = clip(round(src * s), -1, 1) as bf16 (ternary ints)
            w = src_f32.shape[-1]
            for i, p0 in enumerate(range(0, w, 512)):
                pw = min(512, w - p0)
                t1 = tmp_pool.tile([128, 512], f32, tag="wq_t1")
                nc.scalar.activation(
                    out=t1[:, :pw], in_=src_f32[:, p0:p0 + pw],
                    func=ACTF.Identity, bias=magic_col, scale=s_col,
                )
                t2 = tmp_pool.tile([128, 512], f32, tag="wq_t2")
                nc.vector.tensor_scalar(
                    t2[:, :pw], t1[:, :pw], -MAGIC, 1.0, op0=ALU.add, op1=ALU.min
                )
                eng = nc.gpsimd if (i + salt) % 2 else nc.vector
                eng.tensor_scalar(
                    dst_bf16[:, p0:p0 + pw], t2[:, :pw], -1.0, None, op0=ALU.max
                )

        def act_quant_tile(xtile, t2_dst, TQ, fold_col=None, save_clip=None,
                           save_c=None, c_mults=None, salt=0):
            """Quantize one [128, W] token tile into t2_dst (bf16 ints,
            optionally * fold_col).  save_clip/save_c: [128,1] dsts.
            t1 on ACT; t2 alternating DVE/Pool."""
            w = xtile.shape[-1]
            QW = 512
            if not callable(t2_dst):
                _dst_ap = t2_dst
                t2_dst = lambda c0: _dst_ap[:, c0:min(c0 + QW, w)]
            mx = TQ.tile([128, 1], f32, tag="aq_mx")
            nc.vector.tensor_reduce(
                mx, xtile, axis=AX.X, op=ALU.max, apply_absolute_value=True
            )
            clip = TQ.tile([128, 1], f32, tag="aq_clip")
            nc.vector.tensor_scalar(clip, mx, 1e-5, None, op0=ALU.max)
            if save_clip is not None:
                nc.gpsimd.tensor_copy(save_clip, clip)
            sx = TQ.tile([128, 1], f32, tag="aq_sx")
            nc.vector.reciprocal(sx, clip)
            nc.vector.tensor_scalar(sx, sx, 127.0, None, op0=ALU.mult)
            if save_c is not None:
                nc.vector.tensor_scalar(
                    save_c, clip, c_mults[0], c_mults[1],
                    op0=ALU.mult, op1=ALU.mult,
                )
            for i, c0 in enumerate(range(0, w, QW)):
                cw = min(QW, w - c0)
                t1 = TQ.tile([128, QW], f32, tag="aq_t1")
                nc.scalar.activation(
                    out=t1[:, :cw], in_=xtile[:, c0:c0 + cw],
                    func=ACTF.Identity, bias=magic_col, scale=sx,
                )
                eng = nc.gpsimd if (i + salt) % 2 else nc.vector
                if fold_col is not None:
                    eng.tensor_scalar(
                        t2_dst(c0)[:, :cw], t1[:, :cw], -MAGIC, fold_col,
                        op0=ALU.add, op1=ALU.mult,
                    )
                else:
                    eng.tensor_scalar(
                        t2_dst(c0)[:, :cw], t1[:, :cw], -MAGIC, None,
                        op0=ALU.add,
                    )

        # ---- Wq streaming pieces (pass1: abs-sum; pass2: quant+group-sum) --
        wq_state = {}

        def wq_pass1_piece(WL, i):
            if i == 0:
                wq_state["acc"] = PP.tile([128, 1], f32, tag="wacc_q",
                                          name="wacc_q")
                wq_state["tmpc"] = PP.tile([128, 1], f32, tag="wtmp_q",
                                          name="wtmp_q")
            for e in range(4 * i, 4 * i + 4):
                t = WL.tile([128, E], f32, tag="aload")
                (nc.sync if e % 2 else nc.scalar).dma_start(
                    out=t, in_=wqT_d[e * 128:(e + 1) * 128, :])
                abs_acc(wq_state["acc"], wq_state["tmpc"], t, e == 0)
            if i == 3:
                s_q, inv_swq = finish_scale(wq_state["acc"], float(E * E), "q")
                wq_state["s_q"] = s_q
                wq_state["inv_swq"] = inv_swq

        def wq_pass2_piece(WL, WT, wqsumT, i):
            s_q = wq_state["s_q"]
            for e in range(4 * i, 4 * i + 4):
                t = WL.tile([128, E], f32, tag="wq_load")
                (nc.scalar if e % 2 else nc.sync).dma_start(
                    out=t, in_=wqT_d[e * 128:(e + 1) * 128, :])
                for h in range(KH):
                    t1 = WT.tile([128, 512], f32, tag="wq_t1")
                    nc.scalar.activation(
                        out=t1, in_=t[:, h * 512:(h + 1) * 512],
                        func=ACTF.Identity, bias=magic_col, scale=s_q,
                    )
                    t2 = WT.tile([128, 512], f32, tag="wq_t2")
                    nc.vector.tensor_scalar(
                        t2, t1, -MAGIC, 1.0, op0=ALU.add, op1=ALU.min
                    )
                    wqp = WT.tile([128, 512], bf16, tag="wq_p")
                    eng = nc.gpsimd if h % 2 else nc.vector
                    eng.tensor_scalar(wqp, t2, -1.0, None, op0=ALU.max)
                    tmps = WT.tile([128, HD], f32, tag="wq_sumf")
                    nc.vector.tensor_reduce(
                        tmps,
                        wqp.rearrange("p (g d) -> p d g", g=4, d=HD),
                        axis=AX.X,
                        op=ALU.add,
                    )
                    nc.gpsimd.tensor_copy(
                        wqsumT[e][:, h * 128:(h + 1) * 128], tmps
                    )

        # =================== stage 1: quant + projections ===================
        CHUNK = 512

        def quant_chunk(AL, AQ, AC, src_dram, stage, c0, kind, teng):
            """Load+quantize CHUNK tokens at row c0, bounce via DRAM
            stage, return transposed chunk [128, 16, CHUNK] bf16."""
            for ti in range(CHUNK // 128):
                tok0 = c0 + ti * 128
                xt = AL.tile([128, E], f32, tag="aload")
                (nc.sync if ti % 2 else nc.scalar).dma_start(
                    out=xt, in_=src_dram[tok0:tok0 + 128, :])
                jt = tok0 // 128
                t2 = AQ.tile([128, E], bf16, tag="aq_t2")
                if kind == "q":
                    cq = AQ.tile([128, 1], f32, tag="aq_cq")
                    act_quant_tile(
                        xt, t2, AQ, fold_col=cq, save_c=cq,
                        c_mults=(wq_state["inv_swq"], 1.0 / (127.0 * 128.0)),
                        salt=ti,
                    )
                elif kind == "k":
                    act_quant_tile(xt, t2, AQ,
                                   save_clip=clip_k[:, jt:jt + 1], salt=ti)
                else:
                    act_quant_tile(xt, t2, AQ,
                                   save_clip=clip_v[:, jt:jt + 1], salt=ti)
                (nc.scalar if ti % 2 else nc.sync).dma_start(
                    out=stage[tok0:tok0 + 128, :], in_=t2
                )
            chunk = AC.tile([128, 16, CHUNK], bf16, tag="chunk")
            # one xbar transfer: chunk[p, e, t] = stage[c0+t, e*128+p]
            teng.dma_start_transpose(
                out=chunk, in_=stage[c0:c0 + CHUNK, :]
            )
            return chunk

        with tc.tile_pool(name="wkv_int", bufs=1) as WIkv, \
             tc.tile_pool(name="wq_tmp", bufs=2) as WQT, \
             tc.tile_pool(name="aload", bufs=2) as AL, \
             tc.tile_pool(name="aquant", bufs=2) as AQ, \
             tc.tile_pool(name="achunk", bufs=2) as AC:
            wkqT = [WIkv.tile([128, KVE], bf16, tag=f"wkq{e}", name=f"wkq{e}")
                    for e in range(16)]
            wvqT = [WIkv.tile([128, KVE], bf16, tag=f"wvq{e}", name=f"wvq{e}")
                    for e in range(16)]

            # ---- K phase: chunk0 k load FIRST so DVE/ACT start instantly;
            # wk loads right behind on the same HWDGE rings. ----
            with tc.tile_pool(name="k_psum", bufs=2, space="PSUM") as PJ:
                k_chunks = {}

                def emit_k_chunk(ci):
                    c0 = ci * CHUNK
                    chunk = quant_chunk(AL, AQ, AC, k_in, stage_k, c0, "k",
                                        nc.scalar)
                    k_chunks[ci] = chunk

                def emit_k_proj(ci):
                    c0 = ci * CHUNK
                    chunk = k_chunks.pop(ci)
                    for h in range(KH):
                        ps = PJ.tile([128, CHUNK], f32, tag="proj_ps")
                        for e in range(16):
                            nc.tensor.matmul(
                                ps,
                                lhsT=wkqT[e][:, h * 128:(h + 1) * 128],
                                rhs=chunk[:, e, :],
                                start=(e == 0),
                                stop=(e == 15),
                            )
                        nc.vector.tensor_copy(kT[h][:, c0:c0 + CHUNK], ps)

                # chunk0 k-token loads are the very first DMAs emitted
                emit_k_chunk(0)

                # Wk: load + quantize (scoped fp32 staging)
                with tc.tile_pool(name="wk_f32", bufs=1) as WKF:
                    wk_t = [WKF.tile([128, KVE], f32, tag=f"wkf{e}",
                                     name=f"wkf{e}") for e in range(16)]
                    for e in range(16):
                        (nc.sync if e % 2 else nc.scalar).dma_start(
                            out=wk_t[e], in_=wkT_d[e * 128:(e + 1) * 128, :])
                    acc_k = PP.tile([128, 1], f32, tag="wacc_k")
                    tmp_c = PP.tile([128, 1], f32, tag="wtmp_kvo")
                    for e in range(16):
                        abs_acc(acc_k, tmp_c, wk_t[e], e == 0)
                    s_k, inv_swk = finish_scale(acc_k, float(KVE * E), "k")
                    for e in range(16):
                        quant_tile(wkqT[e], wk_t[e], s_k, WQT, salt=e)

                    emit_k_chunk(1)
                    emit_k_proj(0)
                    wq_pass1_piece(AL, 0)
                # Wv: load + quantize while k chunks 2,3 stream
                with tc.tile_pool(name="wv_f32", bufs=1) as WVF:
                    wv_t = [WVF.tile([128, KVE], f32, tag=f"wvf{e}",
                                     name=f"wvf{e}") for e in range(16)]
                    for e in range(16):
                        nc.gpsimd.dma_start(
                            out=wv_t[e], in_=wvT_d[e * 128:(e + 1) * 128, :])
                    emit_k_chunk(2)
                    emit_k_proj(1)
                    wq_pass1_piece(AL, 1)
                    acc_v = PP.tile([128, 1], f32, tag="wacc_v")
                    for e in range(16):
                        abs_acc(acc_v, tmp_c, wv_t[e], e == 0)
                    s_v, inv_swv = finish_scale(acc_v, float(KVE * E), "v")
                    emit_k_chunk(3)
                    emit_k_proj(2)
                    wq_pass1_piece(AL, 2)
                    for e in range(16):
                        quant_tile(wvqT[e], wv_t[e], s_v, WQT, salt=e)
                    emit_k_proj(3)
                    wq_pass1_piece(AL, 3)

            # ck columns for the exp stage
            nc.vector.tensor_scalar(
                ck_all, clip_k, inv_swk, 1.0 / 127.0, op0=ALU.mult, op1=ALU.mult
            )

            # ---- V phase: v chunks + V-proj + Wq pass2 + Q chunks ----
            with tc.tile_pool(name="wqsum", bufs=1) as WIq, \
                 tc.tile_pool(name="wq_load", bufs=2) as WQL, \
                 tc.tile_pool(name="v_psum", bufs=2, space="PSUM") as PJV, \
                 tc.tile_pool(name="q_psum", bufs=1, space="PSUM") as QJ:
                wqsumT = [WIq.tile([128, KVE], bf16, tag=f"wqsum{e}",
                                   name=f"wqsum{e}") for e in range(16)]
                if has_bv:
                    bv_bc = A1.tile([128, KVE], f32, tag="bv_bc")
                    nc.gpsimd.dma_start(out=bv_bc, in_=bcast_ap(bv_d))
                if has_bq:
                    bq_sb = PP.tile([128, 16], f32, tag="bq_sb")
                    nc.sync.dma_start(
                        out=bq_sb, in_=bq_d.rearrange("(j d) -> d j", d=128))
                    bqsum = PP.tile([128, KH], f32, tag="bqsum")
                    nc.vector.tensor_reduce(
                        bqsum,
                        bq_sb.rearrange("p (h g) -> p h g", h=KH, g=4),
                        axis=AX.X,
                        op=ALU.add,
                    )
                    nc.vector.tensor_scalar_mul(bqsum, bqsum, 1.0 / 128.0)

                q_hold = {}

                def emit_query_a(qi):
                    # quant + stage + transpose + first 8 e-tiles of Q-proj
                    # (wqsumT[0:8] are written by wq_pass2 pieces 0-1)
                    c0 = qi * CHUNK
                    chunk = quant_chunk(AL, AQ, AC, q_in, stage_q, c0, "q",
                                        nc.scalar)
                    pss = []
                    for h in range(KH):
                        ps = QJ.tile([128, CHUNK], f32, tag=f"proj_q{h}",
                                     name=f"proj_q{h}_{qi}")
                        for e in range(8):
                            nc.tensor.matmul(
                                ps,
                                lhsT=wqsumT[e][:, h * 128:(h + 1) * 128],
                                rhs=chunk[:, e, :],
                                start=(e == 0),
                                stop=False,
                            )
                        pss.append(ps)
                    q_hold[qi] = (chunk, pss)

                def emit_query_b(qi):
                    # remaining 8 e-tiles + epilogue (needs all wqsumT)
                    c0 = qi * CHUNK
                    chunk, pss = q_hold.pop(qi)
                    for h in range(KH):
                        ps = pss[h]
                        for e in range(8, 16):
                            nc.tensor.matmul(
                                ps,
                                lhsT=wqsumT[e][:, h * 128:(h + 1) * 128],
                                rhs=chunk[:, e, :],
                                start=False,
                                stop=(e == 15),
                            )
                        nc.scalar.activation(
                            out=qT[h][:, c0:c0 + CHUNK],
                            in_=ps,
                            func=ACTF.Identity,
                            bias=(bqsum[:, h:h + 1] if has_bq else 0.0),
                            scale=1.0,
                        )

                for ci in range(S // CHUNK):
                    c0 = ci * CHUNK
                    chunk = quant_chunk(AL, AQ, AC, v_in, stage_v, c0, "v",
                                        nc.sync)
                    # cv for this chunk's 4 token tiles (clips just written)
                    j0 = c0 // 128
                    nc.vector.tensor_scalar(
                        cv_all[:, j0:j0 + 4], clip_v[:, j0:j0 + 4],
                        inv_swv, 1.0 / 127.0, op0=ALU.mult, op1=ALU.mult,
                    )
                    for ti in range(CHUNK // 128):
                        jt = (c0 + ti * 128) // 128
                        ps = PJV.tile([128, KVE], f32, tag="proj_ps_v")
                        for e in range(16):
                            nc.tensor.matmul(
                                ps,
                                lhsT=chunk[:, e, ti * 128:(ti + 1) * 128],
                                rhs=wvqT[e],
                                start=(e == 0),
                                stop=(e == 15),
                            )
                        nc.vector.tensor_scalar(
                            vS[jt], ps, cv_all[:, jt:jt + 1], None, op0=ALU.mult
                        )
                        if has_bv:
                            nc.vector.tensor_add(vS[jt], vS[jt], bv_bc)
                    wq_pass2_piece(WQL, WQT, wqsumT, ci)
                    if ci == 1:
                        emit_query_a(0)
                    elif ci == 3:
                        emit_query_b(0)
                        emit_query_a(1)
                        emit_query_b(1)

        # =================== stage 2: attention =============================
        # local block 0 attends keys < 1024 (true for global blocks 0/1).
        # local block 1 attends all 16 key tiles, unmasked for j < 8.
        A2 = ctx_a2 = tc.tile_pool(name="acts2", bufs=1)
        A2 = ctx_a2.__enter__()
        xT = [A2.tile([128, NT_Q], bf16, tag=f"xT{h}", name=f"xT{h}")
              for h in range(KH)]                  # attention out [dv, n]
        with tc.tile_pool(name="amask", bufs=1) as MP, \
             tc.tile_pool(name="aprobs", bufs=4) as PB, \
             tc.tile_pool(name="azrow", bufs=2) as ZR, \
             tc.tile_pool(name="wo_f32", bufs=1) as WOF, \
             tc.tile_pool(name="wo_tmp", bufs=2) as WOT, \
             tc.tile_pool(name="sim_psum", bufs=3, space="PSUM") as SP_, \
             tc.tile_pool(name="x_psum", bufs=1, space="PSUM") as XP, \
             tc.tile_pool(name="z_psum", bufs=1, space="PSUM") as ZP, \
             tc.tile_pool(name="b_psum", bufs=1, space="PSUM") as BP:
            thr_bc = [MP.tile([128, 512], f32, tag=f"thr{lb}", name=f"thr{lb}")
                      for lb in range(2)]
            for lb in range(2):
                nc.gpsimd.dma_start(out=thr_bc[lb], in_=bcast_ap(thr_d[lb]))
            # masks: lb0 needs j 0..7; lb1 needs only j 8..15
            masks = {}
            for lb, js in ((0, range(8)), (1, range(8, 16))):
                for j in js:
                    m = MP.tile([128, 512], bf16, tag=f"mask_{lb}_{j}",
                                name=f"mask_{lb}_{j}")
                    # mask[p, n] = (thr[lb, n] >= p + 128*j)
                    nc.vector.tensor_scalar(
                        m, thr_bc[lb], sj[:, j:j + 1], None, op0=ALU.is_ge,
                    )
                    masks[lb, j] = m

            # Wo load rides the attention phase (DMA is idle here);
            # quant ops are emitted after head 0 so they fill engine gaps.
            wo_t = [WOF.tile([128, E], f32, tag=f"wof{c}", name=f"wof{c}")
                    for c in range(4)]
            for c in range(4):
                nc.gpsimd.dma_start(out=wo_t[c], in_=woT_d[c * 128:(c + 1) * 128, :])

            def emit_wo_quant():
                acc_o = PP.tile([128, 1], f32, tag="wacc_o")
                tmp_o = PP.tile([128, 1], f32, tag="wtmp_o")
                for c in range(4):
                    abs_acc(acc_o, tmp_o, wo_t[c], c == 0)
                s_o, inv_swo = finish_scale(acc_o, float(E * KVE), "o")
                for c in range(4):
                    quant_tile(woqT[c], wo_t[c], s_o, WOT, salt=c)
                return inv_swo

            inv_swo = None
            for h in range(KH):
                ps_x = [XP.tile([128, 512], f32, tag=f"ps_x{lb}",
                                name=f"ps_x{lb}_{h}") for lb in range(2)]
                ps_z = [ZP.tile([1, 512], f32, tag=f"ps_z{lb}",
                                name=f"ps_z{lb}_{h}") for lb in range(2)]
                for j in range(16):
                    lbs = (0, 1) if j < 8 else (1,)
                    # scores for both blocks share the kT j-tile as lhsT
                    pss = {}
                    for lb in lbs:
                        ps_s = SP_.tile([128, 512], f32, tag="ps_s")
                        nc.tensor.matmul(
                            ps_s,
                            lhsT=kT[h][:, j * 128:(j + 1) * 128],
                            rhs=qT[h][:, lb * 512:(lb + 1) * 512],
                            start=True,
                            stop=True,
                        )
                        pss[lb] = ps_s
                    prb = {}
                    for lb in lbs:
                        probs = PB.tile([128, 512], bf16, tag="probs")
                        nc.scalar.activation(
                            out=probs, in_=pss[lb], func=ACTF.Exp,
                            scale=ck_all[:, j:j + 1],
                        )
                        if (lb, j) in masks:
                            nc.vector.tensor_mul(probs, probs, masks[lb, j])
                        prb[lb] = probs
                    for lb in lbs:
                        nc.tensor.matmul(
                            ps_x[lb],
                            lhsT=vS[j][:, h * 128:(h + 1) * 128],
                            rhs=prb[lb],
                            start=(j == 0),
                            stop=(j == (7 if lb == 0 else 15)),
                        )
                    for lb in lbs:
                        nc.tensor.matmul(
                            ps_z[lb],
                            lhsT=ones_col,
                            rhs=prb[lb],
                            start=(j == 0),
                            stop=(j == (7 if lb == 0 else 15)),
                        )
                for lb in range(2):
                    invz = ZR.tile([1, 512], f32, tag="invz")
                    nc.vector.reciprocal(invz, ps_z[lb])
                    ps_b = BP.tile([128, 512], f32, tag="ps_b")
                    nc.tensor.matmul(ps_b, lhsT=ones_row, rhs=invz,
                                     start=True, stop=True)
                    invz_bc = ZR.tile([128, 512], f32, tag="invz_bc")
                    nc.vector.tensor_copy(invz_bc, ps_b)
                    nc.vector.tensor_mul(
                        xT[h][:, lb * 512:(lb + 1) * 512], ps_x[lb], invz_bc
                    )
                if h == 0:
                    inv_swo = emit_wo_quant()

        # =================== stage 3: layernorm + out quant =================
        xqoT = A2.tile([128, 4, NT_Q], bf16, tag="xqoT")
        if has_ln:
            gamma_bc = A1.tile([128, KVE], f32, tag="gamma_bc")
            beta_bc = A1.tile([128, KVE], f32, tag="beta_bc")
            nc.gpsimd.dma_start(out=gamma_bc, in_=bcast_ap(gamma_d))
            nc.gpsimd.dma_start(out=beta_bc, in_=bcast_ap(beta_d))
        with tc.tile_pool(name="ln", bufs=2) as LN, \
             tc.tile_pool(name="t_psum", bufs=2, space="PSUM") as TP:
            for tb in range(NT_Q // 128):
                xt = LN.tile([128, KVE], f32, tag="ln_x")
                for c in range(4):
                    ps_t = TP.tile([128, 128], bf16, tag="ps_t")
                    nc.tensor.transpose(
                        ps_t, xT[c][:, tb * 128:(tb + 1) * 128], ident
                    )
                    nc.vector.tensor_copy(
                        xt[:, c * 128:(c + 1) * 128], ps_t)
                stats = LN.tile([128, 6], f32, tag="ln_stats")
                nc.vector.bn_stats(out=stats, in_=xt)
                mv = LN.tile([128, 2], f32, tag="ln_mv")
                nc.vector.bn_aggr(out=mv, in_=stats)
                sd = LN.tile([128, 1], f32, tag="ln_sd")
                nc.scalar.activation(
                    out=sd, in_=mv[:, 1:2], func=ACTF.Sqrt, bias=eps_col,
                )
                rstd = LN.tile([128, 1], f32, tag="ln_rstd")
                nc.vector.reciprocal(rstd, sd)
                xn = LN.tile([128, KVE], f32, tag="ln_xn")
                nc.vector.tensor_scalar(
                    xn, xt, mv[:, 0:1], rstd,
                    op0=ALU.subtract, op1=ALU.mult,
                )
                if has_ln:
                    nc.vector.tensor_mul(xn, xn, gamma_bc)
                    nc.vector.tensor_add(xn, xn, beta_bc)
                xqo = LN.tile([128, KVE], bf16, tag="ln_xqo")
                act_quant_tile(
                    xn, xqo, LN, save_c=co_all[:, tb:tb + 1],
                    c_mults=(inv_swo, 1.0 / 127.0), salt=tb,
                )
                nc.scalar.dma_start(
                    out=stage_o[tb * 128:(tb + 1) * 128, :], in_=xqo
                )
            nc.sync.dma_start_transpose(out=xqoT, in_=stage_o[:, :])

        # =================== stage 4: output projection =====================
        if has_bo:
            bo_bc = A1.tile([128, E], f32, tag="bo_bc")
            nc.gpsimd.dma_start(out=bo_bc, in_=bcast_ap(bo_d))
        with tc.tile_pool(name="osb", bufs=3) as OS, \
             tc.tile_pool(name="o_psum", bufs=2, space="PSUM") as OP:
            for tb in range(NT_Q // 128):
                for eb in range(4):
                    ps_o = OP.tile([128, 512], f32, tag="ps_o")
                    for c in range(4):
                        nc.tensor.matmul(
                            ps_o,
                            lhsT=xqoT[:, c, tb * 128:(tb + 1) * 128],
                            rhs=woqT[c][:, eb * 512:(eb + 1) * 512],
                            start=(c == 0),
                            stop=(c == 3),
                        )
                    ot = OS.tile([128, 512], f32, tag="o_t")
                    if has_bo:
                        nc.vector.tensor_scalar(
                            ot, ps_o, co_all[:, tb:tb + 1], None,
                            op0=ALU.mult,
                        )
                        nc.gpsimd.tensor_add(
                            ot, ot, bo_bc[:, eb * 512:(eb + 1) * 512]
                        )
                    else:
                        nc.scalar.activation(
                            out=ot, in_=ps_o, func=ACTF.Identity,
                            scale=co_all[:, tb:tb + 1],
                        )
                    nc.sync.dma_start(
                        out=out_d[tb * 128:(tb + 1) * 128,
                                  eb * 512:(eb + 1) * 512],
                        in_=ot,
                    )

        ctx_a2.__exit__(None, None, None)

    nc.compile()
    return nc


def _get_nc(has_bv=False, has_bq=False, has_bo=False, has_ln=False):
    key = ("nc", has_bq, has_bv, has_bo, has_ln)
    if key not in _CACHE:
        _CACHE[key] = _build(has_bq, has_bv, has_bo, has_ln)
    return _CACHE[key]


def kernel(query, key, value, Wq, bq, Wk, bk, Wv, bv, Wo, bo, gamma, beta):
    from concourse.bass_utils import run_bass_kernel_spmd

    query = np.ascontiguousarray(query, np.float32)
    key = np.ascontiguousarray(key, np.float32)
    value = np.ascontiguousarray(value, np.float32)
    wqT = np.ascontiguousarray(np.asarray(Wq, np.float32).T)
    wkT = np.ascontiguousarray(np.asarray(Wk, np.float32).T)
    wvT = np.ascontiguousarray(np.asarray(Wv, np.float32).T)
    woT = np.ascontiguousarray(np.asarray(Wo, np.float32).T)
    bq = np.ascontiguousarray(bq, np.float32)
    bv_ = np.ascontiguousarray(bv, np.float32)
    bo = np.ascontiguousarray(bo, np.float32)
    gamma = np.ascontiguousarray(gamma, np.float32)
    beta = np.ascontiguousarray(beta, np.float32)

    has_bq = bool(np.any(bq != 0))
    has_bv = bool(np.any(bv_ != 0))
    has_bo = bool(np.any(bo != 0))
    has_ln = bool(np.any(gamma != 1) or np.any(beta != 0))
    nc = _get_nc(has_bv, has_bq, has_bo, has_ln)

    in_maps = []
    for c in range(NCORES):
        b, half = c // 2, c % 2
        blocks = BLKS[half]
        q_rows = np.concatenate(
            [query[b, blk * 512:(blk + 1) * 512, :] for blk in blocks], axis=0
        )
        thr = np.stack(
            [blk * 512 + np.arange(512, dtype=np.float32) for blk in blocks]
        )
        in_maps.append({
            "q_in": np.ascontiguousarray(q_rows),
            "k_in": key[b],
            "v_in": value[b],
            "wqT": wqT, "wkT": wkT, "wvT": wvT, "woT": woT,
            "bq": bq, "bv": bv_, "bo": bo,
            "gamma": gamma, "beta": beta,
            "thr": np.ascontiguousarray(thr),
        })

    res = run_bass_kernel_spmd(nc, in_maps, core_ids=list(range(NCORES)))
    _CACHE["last_result"] = res

    out = np.zeros((B, S, E), np.float32)
    for c in range(NCORES):
        b, half = c // 2, c % 2
        blocks = BLKS[half]
        o = res.results[c]["out"]
        for i, blk in enumerate(blocks):
            out[b, blk * 512:(blk + 1) * 512, :] = o[i * 512:(i + 1) * 512, :]
    return out


# revision 24
# speedup vs baseline: 3.8890x; 1.1190x over previous
"""Trainium2 Bass kernel for BitNet multi-group-query attention.

Problem: nn_BitnetMultiGroupQueryAttention_41755672052100
  B=4, S=2048, E=2048, QH=16, KH=4, HD=128, KVE=512, fp32.

Key algebraic facts exploited (validated in numpy against the reference):
  * The reference einsum SUMS the query-head group axis, so the 4 query heads
    feeding each kv head are pre-summed in the (quantized) weights: the Q
    projection shrinks 4x.
  * softmax needs no max-subtraction here (scores are O(1)); the per-head
    normalizer is computed as a ones-matmul over probs, and per-token
    quantization scales fold into operands as per-partition scalars.
  * BitNet act/weight quantization produces small integers: projections are
    computed exactly with bf16 int-grid operands accumulating in fp32 PSUM.
  * Rounding uses the magic-constant trick ((x+1.5*2^23)-1.5*2^23 = fp32
    round-to-nearest-even); the act clip to [-127,127] never binds since
    scale=127/max|row|.

Layout strategy: activations are quantized in natural [token, feature] tiles,
bounced through a DRAM staging buffer as bf16 and DMA-transposed back in few
large xbar transfers (fp32 cannot use the DMA transpose path, bf16 can).
Attention runs entirely in the transposed [key, query] domain so probabilities
feed the PV matmul directly with no per-tile transposes.  Probabilities, V
and attention outputs are held in bf16 so every matmul runs at the PE's
full bf16 rate (fp32 matmuls cost 4 cycles/row).

Sharding: core c -> batch b=c//2, two 512-token query blocks ({0,3} even
half, {1,2} odd half; balanced causal work).  Every core computes k/v for the
full sequence of its batch.  The program is identical on all 8 cores (SPMD);
per-core behavior differs only through data (causal thresholds fed as input).
The low local block only attends to the first 1024 keys (true for blocks 0/1
on either half), which the program exploits statically.  The high block needs
no mask for the first 1024 keys (always visible), halving mask multiplies.
"""

import os
import sys

for _p in ("/opt/trn_rl_repo", "/root/.axon_site/_ro/trn_rl_repo"):
    if os.path.isdir(_p) and _p not in sys.path:
        sys.path.insert(0, _p)
        break

import numpy as np

B, S, E = 4, 2048, 2048
QH, KH = 16, 4
HD, KVE = 128, 512
NCORES = 8
BLKS = [[0, 3], [1, 2]]        # global 512-token block ids per half
NT_Q = 1024                    # query tokens per core
MAGIC = 12582912.0             # 1.5 * 2**23 : fp32 RNE rounding constant
LN_EPS = 1e-5

_CACHE = {}


def _build(has_bq: bool, has_bv: bool, has_bo: bool, has_ln: bool):
    import concourse.bass as bass
    import concourse.tile as tile
    import concourse.mybir as mybir
    import concourse.bass_isa as bass_isa
    from concourse import bacc
    from concourse.masks import make_identity

    f32 = mybir.dt.float32
    bf16 = mybir.dt.bfloat16
    i32 = mybir.dt.int32
    ALU = mybir.AluOpType
    ACTF = mybir.ActivationFunctionType
    AX = mybir.AxisListType

    nc = bacc.Bacc(None, target_bir_lowering=False)

    # ---------------- DRAM I/O ----------------
    q_in = nc.dram_tensor("q_in", [NT_Q, E], f32, kind="ExternalInput").ap()
    k_in = nc.dram_tensor("k_in", [S, E], f32, kind="ExternalInput").ap()
    v_in = nc.dram_tensor("v_in", [S, E], f32, kind="ExternalInput").ap()
    wqT_d = nc.dram_tensor("wqT", [E, E], f32, kind="ExternalInput").ap()
    wkT_d = nc.dram_tensor("wkT", [E, KVE], f32, kind="ExternalInput").ap()
    wvT_d = nc.dram_tensor("wvT", [E, KVE], f32, kind="ExternalInput").ap()
    woT_d = nc.dram_tensor("woT", [KVE, E], f32, kind="ExternalInput").ap()
    bq_d = nc.dram_tensor("bq", [E], f32, kind="ExternalInput").ap()
    bv_d = nc.dram_tensor("bv", [KVE], f32, kind="ExternalInput").ap()
    bo_d = nc.dram_tensor("bo", [E], f32, kind="ExternalInput").ap()
    gamma_d = nc.dram_tensor("gamma", [KVE], f32, kind="ExternalInput").ap()
    beta_d = nc.dram_tensor("beta", [KVE], f32, kind="ExternalInput").ap()
    thr_d = nc.dram_tensor("thr", [2, 512], f32, kind="ExternalInput").ap()
    out_d = nc.dram_tensor("out", [NT_Q, E], f32, kind="ExternalOutput").ap()

    def bcast_ap(src_ap, parts=128):
        # DMA-replicate a free-only DRAM AP across `parts` partitions
        return bass.AP(
            tensor=src_ap.tensor,
            offset=src_ap.offset,
            ap=[[0, parts]] + list(src_ap.ap),
        )

    with tile.TileContext(nc) as tc:
      with tc.tile_pool(name="persist", bufs=1) as PP, \
           tc.tile_pool(name="acts", bufs=1) as A1, \
           tc.tile_pool(name="wo_int", bufs=1) as WO:
        # ---------- small persistent constants ----------
        ones_col = PP.tile([128, 1], bf16, tag="ones_col")
        nc.vector.memset(ones_col, 1.0)
        ones_row = PP.tile([1, 128], f32, tag="ones_row")
        nc.vector.memset(ones_row, 1.0)
        eps_col = PP.tile([128, 1], f32, tag="eps_col")
        nc.vector.memset(eps_col, LN_EPS)
        magic_col = PP.tile([128, 1], f32, tag="magic_col")
        nc.vector.memset(magic_col, MAGIC)
        nmagic_col = PP.tile([128, 1], f32, tag="nmagic_col")
        nc.vector.memset(nmagic_col, -MAGIC)
        ident = PP.tile([128, 128], bf16, tag="ident")
        make_identity(nc, ident)
        ident_f = PP.tile([128, 128], f32, tag="ident_f")
        make_identity(nc, ident_f)
        sj_i = PP.tile([128, 16], i32, tag="sj_i")
        # sj[p, j] = p + 128*j  (global key index of partition p in s-tile j)
        nc.gpsimd.iota(sj_i, pattern=[[128, 16]], base=0, channel_multiplier=1)
        sj = PP.tile([128, 16], f32, tag="sj")
        nc.vector.tensor_copy(sj, sj_i)

        clip_k = PP.tile([128, 16], f32, tag="clip_k")
        clip_v = PP.tile([128, 16], f32, tag="clip_v")
        ck_all = PP.tile([128, 16], f32, tag="ck_all")
        cv_all = PP.tile([128, 16], f32, tag="cv_all")
        co_all = PP.tile([128, 8], f32, tag="co_all")
        cq_all = PP.tile([128, 8], f32, tag="cq_all")


        # persistent activations
        kT = [A1.tile([128, S], bf16, tag=f"kT{h}", name=f"kT{h}")
              for h in range(KH)]                  # [d, s] int-grid
        vS = [A1.tile([128, KVE], bf16, tag=f"v{j}", name=f"v{j}")
              for j in range(16)]                  # [s, dv] cv-folded, bf16
        qT = [A1.tile([128, NT_Q], bf16, tag=f"qT{h}", name=f"qT{h}")
              for h in range(KH)]                  # [d, n] cq-folded

        woqT = [WO.tile([128, E], bf16, tag=f"woq{c}", name=f"woq{c}")
                for c in range(4)]

        # ---------------- helpers ----------------
        def finish_scale(acc, numel, tag):
            tot = PP.tile([128, 1], f32, tag=f"wtot_{tag}", name=f"wtot_{tag}")
            nc.gpsimd.partition_all_reduce(
                tot, acc, channels=128, reduce_op=bass_isa.ReduceOp.add
            )
            inv_col = PP.tile([128, 1], f32, tag=f"winv_{tag}", name=f"winv_{tag}")
            nc.vector.tensor_scalar(
                inv_col, tot, 1.0 / numel, 1e-5, op0=ALU.mult, op1=ALU.max
            )
            s_col = PP.tile([128, 1], f32, tag=f"ws_{tag}", name=f"ws_{tag}")
            nc.vector.reciprocal(s_col, inv_col)
            return s_col, inv_col

        def abs_acc(acc, tmp, t, first):
            if first:
                nc.vector.tensor_reduce(
                    acc, t, axis=AX.X, op=ALU.add, apply_absolute_value=True
                )
            else:
                nc.vector.tensor_reduce(
                    tmp, t, axis=AX.X, op=ALU.add, apply_absolute_value=True
                )
                nc.vector.tensor_add(acc, acc, tmp)

        def quant_tile(dst_bf16, src_f32, s_col, tmp_pool, salt=0):
            # dst = clip(round(src * s), -1, 1) as bf16 (ternary ints)
            w = src_f32.shape[-1]
            for i, p0 in enumerate(range(0, w, 512)):
                pw = min(512, w - p0)
                t1 = tmp_pool.tile([128, 512], f32, tag="wq_t1")
                nc.scalar.activation(
                    out=t1[:, :pw], in_=src_f32[:, p0:p0 + pw],
                    func=ACTF.Identity, bias=magic_col, scale=s_col,
                )
                t2 = tmp_pool.tile([128, 512], f32, tag="wq_t2")
                nc.vector.tensor_scalar(
                    t2[:, :pw], t1[:, :pw], -MAGIC, 1.0, op0=ALU.add, op1=ALU.min
                )
                eng = nc.gpsimd if (i + salt) % 2 else nc.vector
                eng.tensor_scalar(
                    dst_bf16[:, p0:p0 + pw], t2[:, :pw], -1.0, None, op0=ALU.max
                )

        def act_quant_tile(xtile, t1f, TQ, save_clip=None,
                           save_c=None, c_mults=None):
            """Round one [128, W] token tile: t1f = round_f32(x*sx) + MAGIC
            (exact ints + magic constant; the -MAGIC rides the post-transpose
            PSUM->SBUF copy).  save_clip/save_c: [128,1] dsts."""
            w = xtile.shape[-1]
            QW = 512
            mx = TQ.tile([128, 1], f32, tag="aq_mx")
            nc.vector.tensor_reduce(
                mx, xtile, axis=AX.X, op=ALU.max, apply_absolute_value=True
            )
            clip = TQ.tile([128, 1], f32, tag="aq_clip")
            nc.vector.tensor_scalar(clip, mx, 1e-5, None, op0=ALU.max)
            if save_clip is not None:
                nc.gpsimd.tensor_copy(save_clip, clip)
            sx = TQ.tile([128, 1], f32, tag="aq_sx")
            nc.vector.reciprocal(sx, clip)
            nc.vector.tensor_scalar(sx, sx, 127.0, None, op0=ALU.mult)
            if save_c is not None:
                nc.vector.tensor_scalar(
                    save_c, clip, c_mults[0], c_mults[1],
                    op0=ALU.mult, op1=ALU.mult,
                )
            for i, c0 in enumerate(range(0, w, QW)):
                cw = min(QW, w - c0)
                nc.scalar.activation(
                    out=t1f[:, c0:c0 + cw], in_=xtile[:, c0:c0 + cw],
                    func=ACTF.Identity, bias=magic_col, scale=sx,
                )

        # ---- Wq streaming pieces (pass1: abs-sum; pass2: quant+group-sum) --
        wq_state = {}

        def wq_pass1_piece(WL, AQ_, i):
            if i == 0:
                wq_state["acc"] = PP.tile([128, 1], f32, tag="wacc_q",
                                          name="wacc_q")
                wq_state["tmpc"] = PP.tile([128, 1], f32, tag="wtmp_q",
                                          name="wtmp_q")
            for e in range(4 * i, 4 * i + 4):
                t = WL.tile([128, E], f32, tag="aload")
                (nc.sync if e % 2 else nc.scalar).dma_start(
                    out=t, in_=wqT_d[e * 128:(e + 1) * 128, :])
                scr = AQ_.tile([128, E], bf16, tag="abs_scr", bufs=1)
                acol = AQ_.tile([128, 1], f32, tag="abs_col")
                nc.scalar.activation(out=scr, in_=t, func=ACTF.Abs,
                                     accum_out=acol)
                if e == 0:
                    nc.vector.tensor_copy(wq_state["acc"], acol)
                else:
                    nc.vector.tensor_add(wq_state["acc"], wq_state["acc"],
                                         acol)
            if i == 3:
                s_q, inv_swq = finish_scale(wq_state["acc"], float(E * E), "q")
                wq_state["s_q"] = s_q
                wq_state["inv_swq"] = inv_swq

        def wq_pass2_piece(WL, WT, wqsumT, i):
            s_q = wq_state["s_q"]
            for e in range(4 * i, 4 * i + 4):
                t = WL.tile([128, E], f32, tag="wq_load")
                (nc.scalar if e % 2 else nc.sync).dma_start(
                    out=t, in_=wqT_d[e * 128:(e + 1) * 128, :])
                for h in range(KH):
                    t1 = WT.tile([128, 512], f32, tag="wq_t1")
                    nc.scalar.activation(
                        out=t1, in_=t[:, h * 512:(h + 1) * 512],
                        func=ACTF.Identity, bias=magic_col, scale=s_q,
                    )
                    t2 = WT.tile([128, 512], f32, tag="wq_t2")
                    nc.vector.tensor_scalar(
                        t2, t1, -MAGIC, 1.0, op0=ALU.add, op1=ALU.min
                    )
                    wqp = WT.tile([128, 512], bf16, tag="wq_p")
                    eng = nc.gpsimd if h % 2 else nc.vector
                    eng.tensor_scalar(wqp, t2, -1.0, None, op0=ALU.max)
                    tmps = WT.tile([128, HD], f32, tag="wq_sumf")
                    nc.vector.tensor_reduce(
                        tmps,
                        wqp.rearrange("p (g d) -> p d g", g=4, d=HD),
                        axis=AX.X,
                        op=ALU.add,
                    )
                    nc.gpsimd.tensor_copy(
                        wqsumT[e][:, h * 128:(h + 1) * 128], tmps
                    )

        # =================== stage 1: quant + projections ===================
        CHUNK = 512

        def quant_chunk(AL, AQ, AC, TRP, src_dram, c0, kind):
            """Load+quantize CHUNK tokens at row c0; transpose the quantized
            bf16 int tiles on the PE (4 e-blocks per PSUM bank) into
            chunk[p, e, t] = xq[c0+t, e*128+p]."""
            chunk = AC.tile([128, 16, CHUNK], bf16, tag="chunk")
            for ti in range(CHUNK // 128):
                tok0 = c0 + ti * 128
                xt = AL.tile([128, E], f32, tag="aload")
                (nc.sync if ti % 2 else nc.scalar).dma_start(
                    out=xt, in_=src_dram[tok0:tok0 + 128, :])
                jt = tok0 // 128
                t1f = AQ.tile([128, E], f32, tag="aq_t1f")
                if kind == "q":
                    act_quant_tile(
                        xt, t1f, AQ,
                        save_c=cq_all[:, jt:jt + 1],
                        c_mults=(wq_state["inv_swq"], 1.0 / (127.0 * 128.0)),
                    )
                elif kind == "k":
                    act_quant_tile(xt, t1f, AQ,
                                   save_clip=clip_k[:, jt:jt + 1])
                else:
                    act_quant_tile(xt, t1f, AQ,
                                   save_clip=clip_v[:, jt:jt + 1])
                for g in range(4):
                    ps_tr = TRP.tile([128, 4, 128], f32, tag="ps_tr")
                    for a in range(4):
                        e = g * 4 + a
                        nc.tensor.transpose(
                            ps_tr[:, a, :], t1f[:, e * 128:(e + 1) * 128],
                            ident_f,
                        )
                    dst = chunk[:, g * 4:(g + 1) * 4, ti * 128:(ti + 1) * 128]
                    if g % 2:
                        nc.scalar.activation(out=dst, in_=ps_tr,
                                             func=ACTF.Identity,
                                             bias=nmagic_col)
                    else:
                        nc.vector.tensor_scalar(dst, ps_tr, -MAGIC, None,
                                                op0=ALU.add)
            return chunk

        with tc.tile_pool(name="wkv_int", bufs=1) as WIkv, \
             tc.tile_pool(name="wq_tmp", bufs=2) as WQT, \
             tc.tile_pool(name="aload", bufs=2) as AL, \
             tc.tile_pool(name="aquant", bufs=2) as AQ, \
             tc.tile_pool(name="achunk", bufs=2) as AC, \
             tc.tile_pool(name="tr_psum", bufs=2, space="PSUM") as TRP:
            wkqT = [WIkv.tile([128, KVE], bf16, tag=f"wkq{e}", name=f"wkq{e}")
                    for e in range(16)]
            wvqT = [WIkv.tile([128, KVE], bf16, tag=f"wvq{e}", name=f"wvq{e}")
                    for e in range(16)]

            # ---- K phase: chunk0 k load FIRST so DVE/ACT start instantly;
            # wk loads right behind on the same HWDGE rings. ----
            with tc.tile_pool(name="k_psum", bufs=2, space="PSUM") as PJ:
                k_chunks = {}

                def emit_k_chunk(ci):
                    c0 = ci * CHUNK
                    chunk = quant_chunk(AL, AQ, AC, TRP, k_in, c0, "k")
                    k_chunks[ci] = chunk

                def emit_k_proj(ci):
                    c0 = ci * CHUNK
                    chunk = k_chunks.pop(ci)
                    for h in range(KH):
                        ps = PJ.tile([128, CHUNK], f32, tag="proj_ps")
                        for e in range(16):
                            nc.tensor.matmul(
                                ps,
                                lhsT=wkqT[e][:, h * 128:(h + 1) * 128],
                                rhs=chunk[:, e, :],
                                start=(e == 0),
                                stop=(e == 15),
                            )
                        nc.scalar.activation(out=kT[h][:, c0:c0 + CHUNK],
                                             in_=ps, func=ACTF.Identity)

                # chunk0 k-token loads are the very first DMAs emitted
                emit_k_chunk(0)

                # Wk: load + quantize (scoped fp32 staging)
                with tc.tile_pool(name="wk_f32", bufs=1) as WKF:
                    wk_t = [WKF.tile([128, KVE], f32, tag=f"wkf{e}",
                                     name=f"wkf{e}") for e in range(16)]
                    for e in range(16):
                        (nc.sync if e % 2 else nc.scalar).dma_start(
                            out=wk_t[e], in_=wkT_d[e * 128:(e + 1) * 128, :])
                    acc_k = PP.tile([128, 1], f32, tag="wacc_k")
                    tmp_c = PP.tile([128, 1], f32, tag="wtmp_kvo")
                    for e in range(16):
                        abs_acc(acc_k, tmp_c, wk_t[e], e == 0)
                    s_k, inv_swk = finish_scale(acc_k, float(KVE * E), "k")
                    for e in range(16):
                        quant_tile(wkqT[e], wk_t[e], s_k, WQT, salt=e)

                    emit_k_chunk(1)
                    emit_k_proj(0)
                    wq_pass1_piece(AL, AQ, 0)
                # Wv: load + quantize while k chunks 2,3 stream
                with tc.tile_pool(name="wv_f32", bufs=1) as WVF:
                    wv_t = [WVF.tile([128, KVE], f32, tag=f"wvf{e}",
                                     name=f"wvf{e}") for e in range(16)]
                    for e in range(16):
                        (nc.sync if e % 2 else nc.scalar).dma_start(
                            out=wv_t[e], in_=wvT_d[e * 128:(e + 1) * 128, :])
                    emit_k_chunk(2)
                    emit_k_proj(1)
                    wq_pass1_piece(AL, AQ, 1)
                    acc_v = PP.tile([128, 1], f32, tag="wacc_v")
                    for e in range(16):
                        abs_acc(acc_v, tmp_c, wv_t[e], e == 0)
                    s_v, inv_swv = finish_scale(acc_v, float(KVE * E), "v")
                    emit_k_chunk(3)
                    emit_k_proj(2)
                    wq_pass1_piece(AL, AQ, 2)
                    for e in range(16):
                        quant_tile(wvqT[e], wv_t[e], s_v, WQT, salt=e)
                    emit_k_proj(3)
                    wq_pass1_piece(AL, AQ, 3)

            # ck columns for the exp stage
            nc.vector.tensor_scalar(
                ck_all, clip_k, inv_swk, 1.0 / 127.0, op0=ALU.mult, op1=ALU.mult
            )

            # ---- V phase: v chunks + V-proj + Wq pass2 + Q chunks ----
            with tc.tile_pool(name="wqsum", bufs=1) as WIq, \
                 tc.tile_pool(name="wq_load", bufs=2) as WQL, \
                 tc.tile_pool(name="v_psum", bufs=2, space="PSUM") as PJV, \
                 tc.tile_pool(name="q_psum", bufs=1, space="PSUM") as QJ:
                wqsumT = [WIq.tile([128, KVE], bf16, tag=f"wqsum{e}",
                                   name=f"wqsum{e}") for e in range(16)]
                if has_bv:
                    bv_bc = A1.tile([128, KVE], f32, tag="bv_bc")
                    nc.gpsimd.dma_start(out=bv_bc, in_=bcast_ap(bv_d))
                if has_bq:
                    bq_sb = PP.tile([128, 16], f32, tag="bq_sb")
                    nc.sync.dma_start(
                        out=bq_sb, in_=bq_d.rearrange("(j d) -> d j", d=128))
                    bqsum = PP.tile([128, KH], f32, tag="bqsum")
                    nc.vector.tensor_reduce(
                        bqsum,
                        bq_sb.rearrange("p (h g) -> p h g", h=KH, g=4),
                        axis=AX.X,
                        op=ALU.add,
                    )
                    nc.vector.tensor_scalar_mul(bqsum, bqsum, 1.0 / 128.0)

                q_hold = {}

                def emit_query_a(qi):
                    # quant + stage + transpose + first 8 e-tiles of Q-proj
                    # (wqsumT[0:8] are written by wq_pass2 pieces 0-1)
                    c0 = qi * CHUNK
                    chunk = quant_chunk(AL, AQ, AC, TRP, q_in, c0, "q")
                    pss = []
                    for h in range(KH):
                        ps = QJ.tile([128, CHUNK], f32, tag=f"proj_q{h}",
                                     name=f"proj_q{h}_{qi}")
                        for e in range(8):
                            nc.tensor.matmul(
                                ps,
                                lhsT=wqsumT[e][:, h * 128:(h + 1) * 128],
                                rhs=chunk[:, e, :],
                                start=(e == 0),
                                stop=False,
                            )
                        pss.append(ps)
                    q_hold[qi] = (chunk, pss)

                def emit_query_b(qi):
                    # remaining 8 e-tiles + epilogue (needs all wqsumT).
                    # cq (per-token act scale) is applied here: transpose the
                    # cq columns to a row, broadcast, multiply the raw int
                    # sums.
                    c0 = qi * CHUNK
                    chunk, pss = q_hold.pop(qi)
                    cq_bc = AQ.tile([128, CHUNK], f32, tag="cq_bc")
                    for ti in range(4):
                        jt = 4 * qi + ti
                        trp = TRP.tile([1, 128], f32, tag="ps_tr",
                                       name=f"cqT_{qi}_{ti}")
                        nc.tensor.transpose(
                            trp, cq_all[:, jt:jt + 1], ident_f)
                        cqr = AQ.tile([1, 128], f32, tag="cq_row")
                        nc.vector.tensor_copy(cqr, trp)
                        nc.gpsimd.partition_broadcast(
                            cq_bc[:, ti * 128:(ti + 1) * 128], cqr)
                    for h in range(KH):
                        ps = pss[h]
                        for e in range(8, 16):
                            nc.tensor.matmul(
                                ps,
                                lhsT=wqsumT[e][:, h * 128:(h + 1) * 128],
                                rhs=chunk[:, e, :],
                                start=False,
                                stop=(e == 15),
                            )
                        if has_bq:
                            tq = AQ.tile([128, CHUNK], f32, tag="qT_tmp")
                            nc.vector.tensor_mul(tq, ps, cq_bc)
                            nc.vector.tensor_scalar(
                                qT[h][:, c0:c0 + CHUNK], tq,
                                bqsum[:, h:h + 1], None, op0=ALU.add,
                            )
                        else:
                            nc.vector.tensor_mul(
                                qT[h][:, c0:c0 + CHUNK], ps, cq_bc,
                            )

                for ci in range(S // CHUNK):
                    c0 = ci * CHUNK
                    chunk = quant_chunk(AL, AQ, AC, TRP, v_in, c0, "v")
                    # cv for this chunk's 4 token tiles (clips just written)
                    j0 = c0 // 128
                    nc.vector.tensor_scalar(
                        cv_all[:, j0:j0 + 4], clip_v[:, j0:j0 + 4],
                        inv_swv, 1.0 / 127.0, op0=ALU.mult, op1=ALU.mult,
                    )
                    for ti in range(CHUNK // 128):
                        jt = (c0 + ti * 128) // 128
                        ps = PJV.tile([128, KVE], f32, tag="proj_ps_v")
                        for e in range(16):
                            nc.tensor.matmul(
                                ps,
                                lhsT=chunk[:, e, ti * 128:(ti + 1) * 128],
                                rhs=wvqT[e],
                                start=(e == 0),
                                stop=(e == 15),
                            )
                        nc.vector.tensor_scalar(
                            vS[jt], ps, cv_all[:, jt:jt + 1], None, op0=ALU.mult
                        )
                        if has_bv:
                            nc.vector.tensor_add(vS[jt], vS[jt], bv_bc)
                    wq_pass2_piece(WQL, WQT, wqsumT, ci)
                    if ci == 1:
                        emit_query_a(0)
                    elif ci == 3:
                        emit_query_b(0)
                        emit_query_a(1)
                        emit_query_b(1)

        # =================== stage 2: attention =============================
        # local block 0 attends keys < 1024 (true for global blocks 0/1).
        # local block 1 attends all 16 key tiles, unmasked for j < 8.
        A2 = ctx_a2 = tc.tile_pool(name="acts2", bufs=1)
        A2 = ctx_a2.__enter__()
        xT = [A2.tile([128, NT_Q], bf16, tag=f"xT{h}", name=f"xT{h}")
              for h in range(KH)]                  # attention out [dv, n]
        with tc.tile_pool(name="amask", bufs=1) as MP, \
             tc.tile_pool(name="aprobs", bufs=4) as PB, \
             tc.tile_pool(name="azrow", bufs=2) as ZR, \
             tc.tile_pool(name="wo_f32", bufs=1) as WOF, \
             tc.tile_pool(name="wo_tmp", bufs=2) as WOT, \
             tc.tile_pool(name="sim_psum", bufs=3, space="PSUM") as SP_, \
             tc.tile_pool(name="x_psum", bufs=1, space="PSUM") as XP, \
             tc.tile_pool(name="z_psum", bufs=1, space="PSUM") as ZP, \
             tc.tile_pool(name="b_psum", bufs=1, space="PSUM") as BP:
            thr_bc = [MP.tile([128, 512], f32, tag=f"thr{lb}", name=f"thr{lb}")
                      for lb in range(2)]
            for lb in range(2):
                nc.gpsimd.dma_start(out=thr_bc[lb], in_=bcast_ap(thr_d[lb]))
            # masks: lb0 needs j 0..7; lb1 needs only j 8..15
            masks = {}
            for lb, js in ((0, range(8)), (1, range(8, 16))):
                for j in js:
                    m = MP.tile([128, 512], bf16, tag=f"mask_{lb}_{j}",
                                name=f"mask_{lb}_{j}")
                    # mask[p, n] = (thr[lb, n] >= p + 128*j)
                    nc.vector.tensor_scalar(
                        m, thr_bc[lb], sj[:, j:j + 1], None, op0=ALU.is_ge,
                    )
                    masks[lb, j] = m

            # Wo load rides the attention phase (DMA is idle here);
            # quant ops are emitted after head 0 so they fill engine gaps.
            wo_t = [WOF.tile([128, E], f32, tag=f"wof{c}", name=f"wof{c}")
                    for c in range(4)]
            for c in range(4):
                nc.sync.dma_start(out=wo_t[c], in_=woT_d[c * 128:(c + 1) * 128, :])

            def emit_wo_quant():
                acc_o = PP.tile([128, 1], f32, tag="wacc_o")
                tmp_o = PP.tile([128, 1], f32, tag="wtmp_o")
                for c in range(4):
                    abs_acc(acc_o, tmp_o, wo_t[c], c == 0)
                s_o, inv_swo = finish_scale(acc_o, float(E * KVE), "o")
                for c in range(4):
                    quant_tile(woqT[c], wo_t[c], s_o, WOT, salt=c)
                return inv_swo

            inv_swo = None
            for h in range(KH):
                ps_x = [XP.tile([128, 512], f32, tag=f"ps_x{lb}",
                                name=f"ps_x{lb}_{h}") for lb in range(2)]
                ps_z = [ZP.tile([1, 512], f32, tag=f"ps_z{lb}",
                                name=f"ps_z{lb}_{h}") for lb in range(2)]
                for j in range(16):
                    lbs = (0, 1) if j < 8 else (1,)
                    # scores for both blocks share the kT j-tile as lhsT
                    pss = {}
                    for lb in lbs:
                        ps_s = SP_.tile([128, 512], f32, tag="ps_s")
                        nc.tensor.matmul(
                            ps_s,
                            lhsT=kT[h][:, j * 128:(j + 1) * 128],
                            rhs=qT[h][:, lb * 512:(lb + 1) * 512],
                            start=True,
                            stop=True,
                        )
                        pss[lb] = ps_s
                    prb = {}
                    for lb in lbs:
                        probs = PB.tile([128, 512], bf16, tag="probs")
                        nc.scalar.activation(
                            out=probs, in_=pss[lb], func=ACTF.Exp,
                            scale=ck_all[:, j:j + 1],
                        )
                        if (lb, j) in masks:
                            meng = nc.gpsimd if j % 2 else nc.vector
                            meng.tensor_mul(probs, probs, masks[lb, j])
                        prb[lb] = probs
                    for lb in lbs:
                        nc.tensor.matmul(
                            ps_x[lb],
                            lhsT=vS[j][:, h * 128:(h + 1) * 128],
                            rhs=prb[lb],
                            start=(j == 0),
                            stop=(j == (7 if lb == 0 else 15)),
                        )
                    for lb in lbs:
                        nc.tensor.matmul(
                            ps_z[lb],
                            lhsT=ones_col,
                            rhs=prb[lb],
                            start=(j == 0),
                            stop=(j == (7 if lb == 0 else 15)),
                        )
                for lb in range(2):
                    invz = ZR.tile([1, 512], f32, tag="invz")
                    nc.vector.reciprocal(invz, ps_z[lb])
                    ps_b = BP.tile([128, 512], f32, tag="ps_b")
                    nc.tensor.matmul(ps_b, lhsT=ones_row, rhs=invz,
                                     start=True, stop=True)
                    invz_bc = ZR.tile([128, 512], f32, tag="invz_bc")
                    nc.vector.tensor_copy(invz_bc, ps_b)
                    nc.vector.tensor_mul(
                        xT[h][:, lb * 512:(lb + 1) * 512], ps_x[lb], invz_bc
                    )
                if h == 0:
                    inv_swo = emit_wo_quant()

        # ========= stage 3: layernorm + out quant (transposed space) ========
        # Stats per token via ones-matmuls over xT (feature-major); the rstd
        # factor cancels inside the quantization grid (round((x-mu)*127/amax)
        # is invariant to a per-token scale), so only the per-token output
        # scale co needs it:  co = rstd * amax * inv_swo / 127.
        xqoT = A2.tile([128, 4, NT_Q], bf16, tag="xqoT")
        if has_ln:
            gb_col = A2.tile([128, 4], f32, tag="gb_col")
            bt_col = A2.tile([128, 4], f32, tag="bt_col")
            nc.sync.dma_start(out=gb_col,
                              in_=gamma_d.rearrange("(h d) -> d h", d=128))
            nc.sync.dma_start(out=bt_col,
                              in_=beta_d.rearrange("(h d) -> d h", d=128))
        with tc.tile_pool(name="ln", bufs=2) as LN, \
             tc.tile_pool(name="ln_bc", bufs=1) as LB, \
             tc.tile_pool(name="mu_psum", bufs=2, space="PSUM") as MUP, \
             tc.tile_pool(name="s2_psum", bufs=2, space="PSUM") as S2P, \
             tc.tile_pool(name="co_psum", bufs=2, space="PSUM") as COP:
            for lb in range(2):
                nsl = slice(lb * 512, (lb + 1) * 512)
                ps_mu = MUP.tile([1, 512], f32, tag="ps_mu")
                for h in range(KH):
                    nc.tensor.matmul(ps_mu, lhsT=ones_col, rhs=xT[h][:, nsl],
                                     start=(h == 0), stop=(h == 3))
                ps_s2 = S2P.tile([1, 512], f32, tag="ps_s2")
                for h in range(KH):
                    sqt = LN.tile([128, 512], bf16, tag="ln_sq")
                    nc.vector.tensor_mul(sqt, xT[h][:, nsl], xT[h][:, nsl])
                    nc.tensor.matmul(ps_s2, lhsT=ones_col, rhs=sqt,
                                     start=(h == 0), stop=(h == 3))
                mrow = LN.tile([1, 512], f32, tag="mrow")
                nc.vector.tensor_scalar(mrow, ps_mu, 1.0 / KVE, None,
                                        op0=ALU.mult)
                m2 = LN.tile([1, 512], f32, tag="m2row")
                nc.vector.tensor_mul(m2, mrow, mrow)
                var = LN.tile([1, 512], f32, tag="varrow")
                nc.vector.tensor_scalar(var, ps_s2, 1.0 / KVE, None,
                                        op0=ALU.mult)
                nc.vector.tensor_sub(var, var, m2)
                sd = LN.tile([1, 512], f32, tag="sdrow")
                nc.scalar.activation(out=sd, in_=var, func=ACTF.Sqrt,
                                     bias=eps_col[0:1, :])
                rstd_row = LN.tile([1, 512], f32, tag="rstdrow")
                nc.vector.reciprocal(rstd_row, sd)
                mu_bc = LB.tile([128, 512], f32, tag="mu_bc")
                nc.gpsimd.partition_broadcast(mu_bc, mrow)
                ts = LN.tile([128, 4, 512], f32, tag="tsub")
                for h in range(KH):
                    nc.vector.tensor_sub(ts[:, h, :], xT[h][:, nsl], mu_bc)
                if has_ln:
                    rstd_bc = LB.tile([128, 512], f32, tag="rstd_bc")
                    nc.gpsimd.partition_broadcast(rstd_bc, rstd_row)
                    for h in range(KH):
                        nc.vector.tensor_mul(ts[:, h, :], ts[:, h, :], rstd_bc)
                        nc.vector.tensor_scalar(
                            ts[:, h, :], ts[:, h, :], gb_col[:, h:h + 1],
                            bt_col[:, h:h + 1], op0=ALU.mult, op1=ALU.add,
                        )
                # per-token absmax over all 512 features (partition absmax
                # per h-tile, replicated output, then max across h)
                am = LB.tile([128, 512], f32, tag="am_bc")
                amt = LB.tile([128, 512], f32, tag="am_tmp")
                for h in range(KH):
                    dstm = am if h == 0 else amt
                    nc.gpsimd.partition_all_reduce(
                        dstm, ts[:, h, :], channels=128,
                        reduce_op=bass_isa.ReduceOp.absmax,
                    )
                    if h:
                        nc.vector.tensor_max(am, am, amt)
                nc.vector.tensor_scalar(am, am, 1e-5, None, op0=ALU.max)
                sxb = LB.tile([128, 512], f32, tag="sx_bc")
                nc.vector.reciprocal(sxb, am)
                nc.vector.tensor_scalar(sxb, sxb, 127.0, None, op0=ALU.mult)
                for h in range(KH):
                    t1 = LN.tile([128, 512], f32, tag="ln_t1")
                    nc.vector.tensor_mul(t1, ts[:, h, :], sxb)
                    t1b = LN.tile([128, 512], f32, tag="ln_t1b")
                    nc.scalar.activation(out=t1b, in_=t1, func=ACTF.Identity,
                                         bias=magic_col)
                    eng = nc.gpsimd if h % 2 else nc.vector
                    eng.tensor_scalar(
                        xqoT[:, h, nsl], t1b, -MAGIC, None, op0=ALU.add,
                    )
                # co row -> broadcast -> transpose to per-token columns
                co_r = LN.tile([1, 512], f32, tag="co_row")
                if has_ln:
                    nc.vector.tensor_scalar(
                        co_r, am[0:1, :], inv_swo[0:1, 0:1], 1.0 / 127.0,
                        op0=ALU.mult, op1=ALU.mult,
                    )
                else:
                    nc.vector.tensor_mul(co_r, am[0:1, :], rstd_row)
                    nc.vector.tensor_scalar(
                        co_r, co_r, inv_swo[0:1, 0:1], 1.0 / 127.0,
                        op0=ALU.mult, op1=ALU.mult,
                    )
                co_bc = LB.tile([128, 512], f32, tag="co_bc")
                nc.gpsimd.partition_broadcast(co_bc, co_r)
                for i in range(4):
                    tb = lb * 4 + i
                    ps_c = COP.tile([128, 128], f32, tag="ps_co")
                    nc.tensor.transpose(
                        ps_c, co_bc[:, i * 128:(i + 1) * 128], ident_f)
                    nc.vector.tensor_copy(co_all[:, tb:tb + 1], ps_c[:, 0:1])

        # =================== stage 4: output projection =====================
        if has_bo:
            bo_bc = A1.tile([128, E], f32, tag="bo_bc")
            nc.gpsimd.dma_start(out=bo_bc, in_=bcast_ap(bo_d))
        with tc.tile_pool(name="osb", bufs=3) as OS, \
             tc.tile_pool(name="o_psum", bufs=2, space="PSUM") as OP:
            for tb in range(NT_Q // 128):
                for eb in range(4):
                    ps_o = OP.tile([128, 512], f32, tag="ps_o")
                    for c in range(4):
                        nc.tensor.matmul(
                            ps_o,
                            lhsT=xqoT[:, c, tb * 128:(tb + 1) * 128],
                            rhs=woqT[c][:, eb * 512:(eb + 1) * 512],
                            start=(c == 0),
                            stop=(c == 3),
                        )
                    ot = OS.tile([128, 512], f32, tag="o_t")
                    if has_bo:
                        nc.vector.tensor_scalar(
                            ot, ps_o, co_all[:, tb:tb + 1], None,
                            op0=ALU.mult,
                        )
                        nc.gpsimd.tensor_add(
                            ot, ot, bo_bc[:, eb * 512:(eb + 1) * 512]
                        )
                    else:
                        nc.scalar.activation(
                            out=ot, in_=ps_o, func=ACTF.Identity,
                            scale=co_all[:, tb:tb + 1],
                        )
                    nc.sync.dma_start(
                        out=out_d[tb * 128:(tb + 1) * 128,
                                  eb * 512:(eb + 1) * 512],
                        in_=ot,
                    )

        ctx_a2.__exit__(None, None, None)

    nc.compile()
    return nc


def _get_nc(has_bv=False, has_bq=False, has_bo=False, has_ln=False):
    key = ("nc", has_bq, has_bv, has_bo, has_ln)
    if key not in _CACHE:
        _CACHE[key] = _build(has_bq, has_bv, has_bo, has_ln)
    return _CACHE[key]


def kernel(query, key, value, Wq, bq, Wk, bk, Wv, bv, Wo, bo, gamma, beta):
    from concourse.bass_utils import run_bass_kernel_spmd

    query = np.ascontiguousarray(query, np.float32)
    key = np.ascontiguousarray(key, np.float32)
    value = np.ascontiguousarray(value, np.float32)
    wqT = np.ascontiguousarray(np.asarray(Wq, np.float32).T)
    wkT = np.ascontiguousarray(np.asarray(Wk, np.float32).T)
    wvT = np.ascontiguousarray(np.asarray(Wv, np.float32).T)
    woT = np.ascontiguousarray(np.asarray(Wo, np.float32).T)
    bq = np.ascontiguousarray(bq, np.float32)
    bv_ = np.ascontiguousarray(bv, np.float32)
    bo = np.ascontiguousarray(bo, np.float32)
    gamma = np.ascontiguousarray(gamma, np.float32)
    beta = np.ascontiguousarray(beta, np.float32)

    has_bq = bool(np.any(bq != 0))
    has_bv = bool(np.any(bv_ != 0))
    has_bo = bool(np.any(bo != 0))
    has_ln = bool(np.any(gamma != 1) or np.any(beta != 0))
    nc = _get_nc(has_bv, has_bq, has_bo, has_ln)

    in_maps = []
    for c in range(NCORES):
        b, half = c // 2, c % 2
        blocks = BLKS[half]
        q_rows = np.concatenate(
            [query[b, blk * 512:(blk + 1) * 512, :] for blk in blocks], axis=0
        )
        thr = np.stack(
            [blk * 512 + np.arange(512, dtype=np.float32) for blk in blocks]
        )
        in_maps.append({
            "q_in": np.ascontiguousarray(q_rows),
            "k_in": key[b],
            "v_in": value[b],
            "wqT": wqT, "wkT": wkT, "wvT": wvT, "woT": woT,
            "bq": bq, "bv": bv_, "bo": bo,
            "gamma": gamma, "beta": beta,
            "thr": np.ascontiguousarray(thr),
        })

    res = run_bass_kernel_spmd(nc, in_maps, core_ids=list(range(NCORES)))
    _CACHE["last_result"] = res

    out = np.zeros((B, S, E), np.float32)
    for c in range(NCORES):
        b, half = c // 2, c % 2
        blocks = BLKS[half]
        o = res.results[c]["out"]
        for i, blk in enumerate(blocks):
            out[b, blk * 512:(blk + 1) * 512, :] = o[i * 512:(i + 1) * 512, :]
    return out
